# revision 1
# baseline (speedup 1.0000x reference)
"""GCN (3-layer, PyG GCNConv semantics) on 8 Trainium2 NeuronCores.

Sharding: nodes are partitioned across the 8 cores by destination id
(graph-parallel). Each core aggregates messages for its own node shard; the
per-layer node features ("tables") are replicated via chunked AllGathers so
every core can gather arbitrary source rows with dma_gather (int16 indices,
so the table is split into 4 row-chunks < 2^15 rows; each chunk is exactly
one sub-AllGather region, letting the collectives overlap the producing
compute).

Math (A_hat = D^-1/2 (A+I) D^-1/2): per layer
    out = dinv * (agg of z) [@ W] + b,  z = dinv * (h @ W)
(W folded before aggregation for layers 1/2, after for layer 3), where
    agg_n = z_n + sum_{e: dst=n} z_src.

Device pipeline per core: L0 computes z1^T per 256-node group
(feature-major); each aggregation layer gathers 128-edge tiles from the
table, builds a selection matrix S[e, j] = (dstoff[e] == j) on DVE and
accumulates psum[64, 256] with f32r matmuls (lhsT = gathered rows,
rhs = S); group epilogues run feature-major, and PE transposes convert
back to node-major shard rows for the next table.
"""
import sys
sys.path.insert(0, '/opt/trn_rl_repo')

from contextlib import ExitStack

import numpy as np

from concourse import bass, bacc, tile, mybir, library_config
from concourse.bass_utils import run_bass_kernel_spmd
from concourse.masks import make_identity

# ---- problem constants (hardcoded) ----
N_NODES = 100000
IN_DIM, HID_DIM, OUT_DIM = 128, 64, 7
N_CORES = 8
NS_RAW = N_NODES // N_CORES          # 12500 real nodes per core
P = 128
G = 256                              # nodes per aggregation group
NGRP = 49                            # 12544 / 256
NS = NGRP * G                        # 12544 padded shard size
V = NS * N_CORES                     # 100352 table rows
SUB_GRPS = [13, 12, 12, 12]          # groups per sub-AllGather / idx chunk
N_CHUNK = len(SUB_GRPS)
WAVE_SPLITS = {13: [5, 4, 4], 12: [4, 4, 4]}
NI_MAX = 1024                        # dma_gather idxs per instruction cap

f32 = mybir.dt.float32
f32r = mybir.dt.float32r
i16 = mybir.dt.int16

SUB_G0 = np.cumsum([0] + SUB_GRPS)           # group start per sub
SUB_ROWS = [g * G for g in SUB_GRPS]         # shard rows per sub
SUB_R0 = np.cumsum([0] + SUB_ROWS)           # shard row start per sub
CHUNK_ROWS = [r * N_CORES for r in SUB_ROWS]  # table rows per chunk
CHUNK_BASE = np.cumsum([0] + CHUNK_ROWS)
SUB_OF_GROUP = np.concatenate(
    [np.full(n, s, dtype=np.int64) for s, n in enumerate(SUB_GRPS)])

# waves: list of (sub, [groups])
WAVES = []
for s in range(N_CHUNK):
    g0 = SUB_G0[s]
    for w in WAVE_SPLITS[SUB_GRPS[s]]:
        WAVES.append((s, list(range(g0, g0 + w))))
        g0 += w
WGRP_MAX = max(len(wg) for _, wg in WAVES)


def _host_prep(edge_index):
    """Partition/permute/pad the graph into a static structure shared by all
    cores (cores differ only in input data, not program shape)."""
    src = np.asarray(edge_index[0], dtype=np.int64)
    dst = np.asarray(edge_index[1], dtype=np.int64)
    deg = np.bincount(dst, minlength=N_NODES).astype(np.float64) + 1.0
    dinv = (1.0 / np.sqrt(deg)).astype(np.float32)

    core_of = dst // NS_RAW
    perms = []
    for c in range(N_CORES):
        sel = core_of == c
        dl = (dst[sel] - c * NS_RAW).astype(np.int64)
        cnt = np.bincount(dl, minlength=NS)
        order = np.argsort(-cnt, kind='stable')
        gload = np.zeros(NGRP, dtype=np.int64)
        gfill = np.zeros(NGRP, dtype=np.int64)
        perm = np.empty(NS, dtype=np.int64)
        for node in order:
            cand = np.where(gfill < G)[0]
            gsel = cand[np.argmin(gload[cand])]
            perm[node] = gsel * G + gfill[gsel]
            gfill[gsel] += 1
            gload[gsel] += cnt[node]
        perms.append(perm)

    def trow_chunkidx(nodes):
        """global node id -> (chunk, in-chunk row)"""
        c = nodes // NS_RAW
        loc = np.empty(len(nodes), dtype=np.int64)
        for cc in range(N_CORES):
            m = c == cc
            loc[m] = perms[cc][nodes[m] % NS_RAW]
        g = loc // G
        s = SUB_OF_GROUP[g]
        inrow = c * np.array(SUB_ROWS)[s] + (loc - SUB_R0[s])
        return s, inrow

    src_ch, src_row = trow_chunkidx(src)

    # per-core (group, chunk) runs
    run_lens = np.zeros((N_CORES, NGRP, N_CHUNK), dtype=np.int64)
    edge_lists = []
    for c in range(N_CORES):
        sel = core_of == c
        rows_c, ch_c = src_row[sel], src_ch[sel]
        d_new = perms[c][(dst[sel] - c * NS_RAW)]
        grp, off = d_new // G, d_new % G
        runs = {}
        for g in range(NGRP):
            gm = grp == g
            for k in range(N_CHUNK):
                m = gm & (ch_c == k)
                runs[(g, k)] = (rows_c[m], off[m])
                run_lens[c, g, k] = m.sum()
        edge_lists.append(runs)

    # static tiles per (group, chunk): max over cores
    t_arr = np.ceil(run_lens.max(axis=0) / P).astype(np.int64)  # [NGRP, N_CHUNK]

    # flat tile order + instruction plan (identical for all cores)
    instr_plan = []   # (wave_idx, chunk, tile_off_in_wavechunk, n_idx, idx_col0)
    tiles_meta = []   # flat: (wave_idx, chunk, group, j)
    wave_tiles = []   # tiles per wave
    idx_cols = 0
    for wi, (s, wg) in enumerate(WAVES):
        wt = 0
        for k in range(N_CHUNK):
            slots = int(t_arr[wg, k].sum()) * P
            done = 0
            while done < slots:
                ni = min(NI_MAX, slots - done)
                instr_plan.append((wi, k, done // P, ni, idx_cols))
                idx_cols += ni // 16
                done += ni
            for g in wg:
                for j in range(int(t_arr[g, k])):
                    tiles_meta.append((wi, k, g, j))
                wt += int(t_arr[g, k])
        wave_tiles.append(wt)
    tt = len(tiles_meta)

    # flat slot base of each (wave, chunk) region
    wc_tilebase = {}
    ti = 0
    for wi, (s, wg) in enumerate(WAVES):
        for k in range(N_CHUNK):
            wc_tilebase[(wi, k)] = ti
            ti += int(t_arr[wg, k].sum())

    idx_arrs, doff_arrs = [], []
    for c in range(N_CORES):
        runs = edge_lists[c]
        flat_idx = np.zeros(tt * P, dtype=np.int16)
        flat_off = np.full(tt * P, -1.0, dtype=np.float32)
        pos = 0
        for (wi, k, g, j) in tiles_meta:
            rows, offs = runs[(g, k)]
            a, b = j * P, min((j + 1) * P, len(rows))
            n = max(0, b - a)
            if n > 0:
                flat_idx[pos:pos + n] = rows[a:b].astype(np.int16)
                flat_off[pos:pos + n] = offs[a:b].astype(np.float32)
            pos += P
        idx_wrapped = np.zeros((P, idx_cols), dtype=np.int16)
        for (wi, k, toff, ni, col0) in instr_plan:
            s0 = (wc_tilebase[(wi, k)] + toff) * P
            blk = flat_idx[s0:s0 + ni].reshape(ni // 16, 16).T
            idx_wrapped[:, col0:col0 + ni // 16] = np.tile(blk, (8, 1))
        idx_arrs.append(idx_wrapped)
        doff_arrs.append(flat_off.reshape(tt, P).T.copy())

    return dict(
        dinv=dinv, perms=perms, t_arr=t_arr, instr_plan=instr_plan,
        tiles_meta=tiles_meta, tt=tt, wave_tiles=wave_tiles,
        wc_tilebase=wc_tilebase, idx_arrs=idx_arrs, doff_arrs=doff_arrs,
        idx_cols=idx_cols,
    )


def _build_program(S, repeat=1, ag_mode='collective'):
    t_arr, instr_plan, tt, idx_cols = (
        S['t_arr'], S['instr_plan'], S['tt'], S['idx_cols'])
    wave_tiles, wc_tilebase = S['wave_tiles'], S['wc_tilebase']
    mw_tiles_max = max(wave_tiles)

    nc = bacc.Bacc("TRN2", target_bir_lowering=False, debug=False,
                   num_devices=N_CORES)

    xT_d = nc.dram_tensor("xT", [P, NS], f32r, kind="ExternalInput")
    dinvrep_d = nc.dram_tensor("dinvrep", [HID_DIM, NS], f32, kind="ExternalInput")
    idx_d = nc.dram_tensor("idx", [P, idx_cols], i16, kind="ExternalInput")
    doff_d = nc.dram_tensor("doff", [P, tt], f32, kind="ExternalInput")
    W1_d = nc.dram_tensor("W1", [IN_DIM, HID_DIM], f32r, kind="ExternalInput")
    W2_d = nc.dram_tensor("W2", [HID_DIM, HID_DIM], f32r, kind="ExternalInput")
    W3_d = nc.dram_tensor("W3", [HID_DIM, 8], f32r, kind="ExternalInput")
    b1_d = nc.dram_tensor("b1", [HID_DIM, 1], f32, kind="ExternalInput")
    b2_d = nc.dram_tensor("b2", [HID_DIM, 1], f32, kind="ExternalInput")
    b3_d = nc.dram_tensor("b3", [8, 1], f32, kind="ExternalInput")
    out_d = nc.dram_tensor("out_shard", [NS, 8], f32, kind="ExternalOutput")

    nc.gpsimd.load_library(library_config.mlp)

    with tile.TileContext(nc) as tc:
        stack = ExitStack()
        zsh = [tc.tile([NS, HID_DIM], f32r, space="DRAM", name=f"zsh{i}")[0]
               for i in range(3)]
        dramp = stack.enter_context(
            tc.tile_pool(name="dramp", bufs=1, space="DRAM"))
        def alloc_tables(rep):
            return [[dramp.tile([CHUNK_ROWS[k], HID_DIM], f32r,
                                addr_space="Shared",
                                name=f"table{rep}_{i}_{k}",
                                tag=f"table{rep}_{i}_{k}")
                     for k in range(N_CHUNK)] for i in range(3)]
        const = stack.enter_context(tc.tile_pool(name="const", bufs=1))

        R_i = const.tile([P, G], mybir.dt.int32)
        nc.gpsimd.iota(R_i[:], pattern=[[1, G]], base=0, channel_multiplier=0)
        R_f = const.tile([P, G], f32)
        nc.vector.tensor_copy(out=R_f[:], in_=R_i[:])
        ident = const.tile([P, P], f32)
        make_identity(nc, ident[:])
        ident_r = const.tile([P, P], f32r)
        nc.vector.tensor_copy(out=ident_r[:], in_=ident[:])

        W1_t = const.tile([IN_DIM, HID_DIM], f32r)
        nc.sync.dma_start(out=W1_t[:], in_=W1_d[:])
        W2_t = const.tile([HID_DIM, HID_DIM], f32r)
        nc.sync.dma_start(out=W2_t[:], in_=W2_d[:])
        W3_t = const.tile([HID_DIM, 8], f32r)
        nc.sync.dma_start(out=W3_t[:], in_=W3_d[:])
        b1_t = const.tile([HID_DIM, 1], f32)
        nc.sync.dma_start(out=b1_t[:], in_=b1_d[:])
        b2_t = const.tile([HID_DIM, 1], f32)
        nc.sync.dma_start(out=b2_t[:], in_=b2_d[:])
        b3_t = const.tile([8, 1], f32)
        nc.sync.dma_start(out=b3_t[:], in_=b3_d[:])
        idx_t = const.tile([P, idx_cols], i16)
        nc.sync.dma_start(out=idx_t[:], in_=idx_d[:])
        doff_t = const.tile([P, tt], f32)
        nc.sync.dma_start(out=doff_t[:], in_=doff_d[:])

        zTd = [tc.tile([HID_DIM, NS], f32r, space="DRAM", name=f"zTd{i}")[0]
               for i in range(2)]

        sbuf = stack.enter_context(tc.tile_pool(name="sbuf", bufs=3))
        spool = stack.enter_context(tc.tile_pool(name="spool", bufs=6))
        wavep = stack.enter_context(tc.tile_pool(name="wavep", bufs=2))
        znodep = stack.enter_context(tc.tile_pool(name="znodep", bufs=2))
        psum_agg = stack.enter_context(
            tc.tile_pool(name="psum_agg", bufs=3, space="PSUM"))
        psum_mm2 = stack.enter_context(
            tc.tile_pool(name="psum_mm2", bufs=2, space="PSUM"))
        psum_tr = stack.enter_context(
            tc.tile_pool(name="psum_tr", bufs=2, space="PSUM"))

        def load_dvw(wg):
            w0, wn = wg[0] * G, len(wg) * G
            dvw = wavep.tile([HID_DIM, wn], f32, tag="dvw",
                             padded_shape=[HID_DIM, WGRP_MAX * G])
            nc.sync.dma_start(out=dvw[:], in_=dinvrep_d[:, w0:w0 + wn])
            return dvw

        def store_wave_fm(zcw, wg, fdim, node_dram, zT_target):
            """Batch-transpose the feature-major wave tile [fdim, wn] into
            node-major [wn, fdim] rows of node_dram; also stash feature-major
            into zT_target if given."""
            w0, wn = wg[0] * G, len(wg) * G
            nch = wn // P
            for blk0 in range(0, nch, 8):
                nb = min(8, nch - blk0)
                ptr = psum_tr.tile([P, nb * fdim], f32r, tag="ptr",
                                   padded_shape=[P, 8 * HID_DIM])
                for i in range(nb):
                    nc.tensor.transpose(
                        out=ptr[:, i * fdim:(i + 1) * fdim],
                        in_=zcw[:fdim, (blk0 + i) * P:(blk0 + i + 1) * P],
                        identity=ident_r[:fdim, :fdim])
                zn = znodep.tile([P, nb * fdim], f32r, tag="zn",
                                 padded_shape=[P, 8 * HID_DIM])
                nc.vector.tensor_copy(out=zn[:], in_=ptr[:])
                dst = node_dram[w0 + blk0 * P: w0 + (blk0 + nb) * P, :]
                src_ap = zn[:] if node_dram is not out_d else zn[:].bitcast(f32)
                nc.scalar.dma_start(
                    out=dst.rearrange("(c p) f -> p c f", p=P),
                    in_=src_ap.rearrange("p (c f) -> p c f", f=fdim))
            if zT_target is not None:
                nc.scalar.dma_start(out=zT_target[:, w0:w0 + wn], in_=zcw[:])

        def sub_allgather(zsh_t, table_t, s):
            r0, rn = SUB_R0[s], SUB_ROWS[s]
            if ag_mode == 'local':
                nc.scalar.dma_start(out=table_t[s][0:rn, :],
                                    in_=zsh_t[r0:r0 + rn, :])
                return
            nc.gpsimd.collective_compute(
                "AllGather", mybir.AluOpType.bypass,
                replica_groups=[list(range(N_CORES))],
                ins=[zsh_t[r0:r0 + rn, :]],
                outs=[table_t[s][:]])

        for _rep in range(repeat):
            tables = alloc_tables(_rep)
            # ---------- L0: z1 = dinv .* (x @ W1), feature-major ----------
            for wi, (s, wg) in enumerate(WAVES):
                w0, wn = wg[0] * G, len(wg) * G
                xw = wavep.tile([P, wn], f32r, tag="xw",
                                padded_shape=[P, WGRP_MAX * G])
                nc.sync.dma_start(out=xw[:], in_=xT_d[:, w0:w0 + wn])
                dvw = load_dvw(wg)
                zcw = wavep.tile([HID_DIM, wn], f32r, tag="zcw",
                                 padded_shape=[HID_DIM, WGRP_MAX * G])
                for g in wg:
                    c0 = (g - wg[0]) * G
                    ps = psum_agg.tile([HID_DIM, G], f32, tag="ps")
                    nc.tensor.matmul(out=ps[:], lhsT=W1_t[:],
                                     rhs=xw[:, c0:c0 + G],
                                     start=True, stop=True)
                    nc.vector.tensor_tensor(
                        out=zcw[:, c0:c0 + G], in0=ps[:],
                        in1=dvw[:, c0:c0 + G], op=mybir.AluOpType.mult)
                store_wave_fm(zcw, wg, HID_DIM, zsh[0], zTd[0])
                if wi + 1 == len(WAVES) or WAVES[wi + 1][0] != s:
                    sub_allgather(zsh[0], tables[0], s)

            # ---------- aggregation layers ----------
            def agg_layer(layer, table, zT_in, zT_out_d, W_next, bias_t,
                          final=False):
                for wi, (s, wg) in enumerate(WAVES):
                    w0, wn = wg[0] * G, len(wg) * G
                    wtiles = wave_tiles[wi]
                    mw = wavep.tile([P, wtiles, HID_DIM], f32r, tag="mw",
                                    padded_shape=[P, mw_tiles_max, HID_DIM])
                    wave_t0 = wc_tilebase[(wi, 0)]
                    for (wi2, k, toff, ni, col0) in instr_plan:
                        if wi2 != wi:
                            continue
                        ck = wc_tilebase[(wi, k)] - wave_t0
                        nc.gpsimd.dma_gather(
                            out_ap=mw[:, ck + toff: ck + toff + ni // P, :],
                            in_ap=table[k][:],
                            idxs_ap=idx_t[:, col0: col0 + ni // 16],
                            num_idxs=ni, num_idxs_reg=ni, elem_size=HID_DIM,
                            single_packet=True,
                        )
                    zsw = wavep.tile([HID_DIM, wn], f32r, tag="zsw",
                                     padded_shape=[HID_DIM, WGRP_MAX * G])
                    nc.sync.dma_start(out=zsw[:], in_=zT_in[:, w0:w0 + wn])
                    dvw = load_dvw(wg)
                    if final:
                        zcw = wavep.tile([8, wn], f32r, tag="ocw",
                                         padded_shape=[8, WGRP_MAX * G])
                    else:
                        zcw = wavep.tile([HID_DIM, wn], f32r, tag="zcw",
                                         padded_shape=[HID_DIM, WGRP_MAX * G])
                    for gi, g in enumerate(wg):
                        ps = psum_agg.tile([HID_DIM, G], f32, tag="ps")
                        n_mm = int(t_arr[g].sum())
                        mm_i = 0
                        for k in range(N_CHUNK):
                            ck = wc_tilebase[(wi, k)] - wave_t0
                            jbase = int(t_arr[wg[0]:g, k].sum())
                            for j in range(int(t_arr[g, k])):
                                wt = ck + jbase + j
                                ft = wave_t0 + wt if k == 0 else (
                                    wc_tilebase[(wi, k)] + jbase + j)
                                St = spool.tile([P, G], f32r, tag="St")
                                nc.vector.tensor_scalar(
                                    out=St[:], in0=R_f[:],
                                    scalar1=doff_t[:, ft:ft + 1], scalar2=None,
                                    op0=mybir.AluOpType.is_equal)
                                nc.tensor.matmul(
                                    out=ps[:], lhsT=mw[:, wt, :], rhs=St[:],
                                    start=(mm_i == 0), stop=(mm_i == n_mm - 1))
                                mm_i += 1
                        # ---- epilogue for group g ----
                        c0 = (g - wg[0]) * G
                        c1 = c0 + G
                        e1 = sbuf.tile([HID_DIM, G], f32, tag="e1")
                        nc.vector.tensor_tensor(out=e1[:], in0=ps[:],
                                                in1=zsw[:, c0:c1],
                                                op=mybir.AluOpType.add)
                        if final:
                            e2 = sbuf.tile([HID_DIM, G], f32r, tag="e2")
                            nc.vector.tensor_tensor(out=e2[:], in0=e1[:],
                                                    in1=dvw[:, c0:c1],
                                                    op=mybir.AluOpType.mult)
                            po = psum_mm2.tile([8, G], f32, tag="po")
                            nc.tensor.matmul(out=po[:], lhsT=W3_t[:],
                                             rhs=e2[:], start=True, stop=True)
                            nc.vector.tensor_scalar(
                                out=zcw[:, c0:c1], in0=po[:],
                                scalar1=b3_t[:, :1],
                                scalar2=None, op0=mybir.AluOpType.add)
                        else:
                            e2 = sbuf.tile([HID_DIM, G], f32, tag="e2")
                            nc.vector.tensor_tensor(out=e2[:], in0=e1[:],
                                                    in1=dvw[:, c0:c1],
                                                    op=mybir.AluOpType.mult)
                            hT = sbuf.tile([HID_DIM, G], f32r, tag="hT")
                            nc.vector.tensor_scalar(
                                out=hT[:], in0=e2[:], scalar1=bias_t[:, :1],
                                scalar2=0.0, op0=mybir.AluOpType.add,
                                op1=mybir.AluOpType.max)
                            if W_next is not None:
                                po = psum_mm2.tile([HID_DIM, G], f32, tag="po")
                                nc.tensor.matmul(out=po[:], lhsT=W_next[:],
                                                 rhs=hT[:], start=True,
                                                 stop=True)
                                nc.vector.tensor_tensor(
                                    out=zcw[:, c0:c1], in0=po[:],
                                    in1=dvw[:, c0:c1],
                                    op=mybir.AluOpType.mult)
                            else:
                                nc.vector.tensor_tensor(
                                    out=zcw[:, c0:c1], in0=hT[:],
                                    in1=dvw[:, c0:c1],
                                    op=mybir.AluOpType.mult)
                    if final:
                        store_wave_fm(zcw, wg, 8, out_d, None)
                    else:
                        store_wave_fm(zcw, wg, HID_DIM, zsh[layer],
                                      zT_out_d)
                        if wi + 1 == len(WAVES) or WAVES[wi + 1][0] != s:
                            sub_allgather(zsh[layer], tables[layer], s)

            agg_layer(1, tables[0], zTd[0], zTd[1], W2_t, b1_t)
            agg_layer(2, tables[1], zTd[1], zTd[0], None, b2_t)
            agg_layer(3, tables[2], zTd[0], None, None, b3_t, final=True)
        stack.close()

    nc.finalize()
    return nc


def _make_in_maps(S, x, W1, b1, W2, b2, W3, b3):
    dinv = S['dinv']
    W3p = np.zeros((HID_DIM, 8), np.float32)
    W3p[:, :OUT_DIM] = W3
    b3p = np.zeros((8, 1), np.float32)
    b3p[:OUT_DIM, 0] = b3
    in_maps = []
    for c in range(N_CORES):
        perm = S['perms'][c]
        xs = np.zeros((NS, IN_DIM), np.float32)
        dv = np.ones(NS, np.float32)
        xs[perm[:NS_RAW]] = x[c * NS_RAW:(c + 1) * NS_RAW]
        dv[perm[:NS_RAW]] = dinv[c * NS_RAW:(c + 1) * NS_RAW]
        in_maps.append({
            "xT": np.ascontiguousarray(xs.T),
            "dinvrep": np.ascontiguousarray(
                np.broadcast_to(dv[None, :], (HID_DIM, NS))),
            "idx": S['idx_arrs'][c],
            "doff": S['doff_arrs'][c],
            "W1": W1, "W2": W2, "W3": W3p,
            "b1": b1.reshape(-1, 1), "b2": b2.reshape(-1, 1), "b3": b3p,
        })
    return in_maps


_LAST = {}


def kernel(x, edge_index, W1, b1, W2, b2, W3, b3):
    x = np.asarray(x, dtype=np.float32)
    W1 = np.asarray(W1, dtype=np.float32)
    W2 = np.asarray(W2, dtype=np.float32)
    W3 = np.asarray(W3, dtype=np.float32)
    b1 = np.asarray(b1, dtype=np.float32)
    b2 = np.asarray(b2, dtype=np.float32)
    b3 = np.asarray(b3, dtype=np.float32)

    S = _host_prep(edge_index)
    nc = _build_program(S)
    in_maps = _make_in_maps(S, x, W1, b1, W2, b2, W3, b3)

    res = run_bass_kernel_spmd(nc, in_maps, core_ids=list(range(N_CORES)))

    _LAST['S'] = S
    _LAST['in_maps'] = in_maps

    out = np.empty((N_NODES, OUT_DIM), np.float32)
    for c in range(N_CORES):
        shard = res.results[c]["out_shard"]       # [NS, 8]
        perm = S['perms'][c]
        out[c * NS_RAW:(c + 1) * NS_RAW] = shard[perm[:NS_RAW], :OUT_DIM]
    return out


def measure_exec_ns(repeats=(1, 5), iters=6, ag_mode='collective'):
    """Estimate HW exec time by building R-times-repeated variants of the
    full pipeline and differencing pipelined wall-clock."""
    import time
    import jax
    from jax.sharding import Mesh, PartitionSpec, NamedSharding
    from jax.experimental.shard_map import shard_map
    from concourse import bass2jax
    from concourse.bass2jax import _bass_exec_p, install_neuronx_cc_hook

    S, in_maps = _LAST['S'], _LAST['in_maps']
    install_neuronx_cc_hook()
    per_call = {}
    for R in repeats:
        nc = _build_program(S, repeat=R, ag_mode=ag_mode)
        partition_name = (nc.partition_id_tensor.name
                          if nc.partition_id_tensor else None)
        in_names, out_names, out_avals, zero_outs = [], [], [], []
        for alloc in nc.m.functions[0].allocations:
            if not isinstance(alloc, mybir.MemoryLocationSet):
                continue
            name = alloc.memorylocations[0].name
            if alloc.kind == "ExternalInput":
                if name != partition_name:
                    in_names.append(name)
            elif alloc.kind == "ExternalOutput":
                out_names.append(name)
                shape = tuple(alloc.tensor_shape)
                dtype = mybir.dt.np(alloc.dtype)
                out_avals.append(jax.core.ShapedArray(shape, dtype))
                zero_outs.append(np.zeros(shape, dtype))
        all_in = list(in_names) + list(out_names)
        if partition_name:
            all_in.append(partition_name)

        def _body(*args, _nc=nc, _avals=tuple(out_avals), _in=tuple(all_in),
                  _out=tuple(out_names)):
            operands = list(args)
            operands.append(bass2jax.partition_id_tensor())
            return tuple(_bass_exec_p.bind(
                *operands, out_avals=_avals, in_names=_in, out_names=_out,
                lowering_input_output_aliases=(), sim_require_finite=True,
                sim_require_nnan=True, nc=_nc))

        devices = jax.devices()[:N_CORES]
        mesh = Mesh(np.asarray(devices), ("core",))
        nsp = len(in_names) + len(zero_outs)
        sharded = jax.jit(shard_map(
            _body, mesh=mesh, in_specs=(PartitionSpec("core"),) * nsp,
            out_specs=(PartitionSpec("core"),) * len(out_names),
            check_rep=False), keep_unused=True)
        args = [np.concatenate([np.asarray(in_maps[c][n]) for c in
                                range(N_CORES)], axis=0) for n in in_names]
        args += [np.zeros((N_CORES * z.shape[0], *z.shape[1:]), z.dtype)
                 for z in zero_outs]
        sh = NamedSharding(mesh, PartitionSpec("core"))
        args = [jax.device_put(a, sh) for a in args]
        outs = sharded(*args)
        jax.block_until_ready(outs)
        best = None
        for _ in range(iters):
            t0 = time.perf_counter()
            got = [sharded(*args) for _ in range(4)]
            jax.block_until_ready(got)
            dt = (time.perf_counter() - t0) / 4
            best = dt if best is None else min(best, dt)
        per_call[R] = best
    r0, r1 = repeats
    est = (per_call[r1] - per_call[r0]) / (r1 - r0)
    return max(1, int(est * 1e9))



# revision 10
# speedup vs baseline: 1.3741x; 1.3741x over previous
"""GCN (3-layer, PyG GCNConv semantics) on 8 Trainium2 NeuronCores.

Sharding: nodes are partitioned across the 8 cores by destination id
(graph-parallel). Each core aggregates messages for its own node shard; the
per-layer node features ("tables") are replicated via chunked AllGathers so
every core can gather arbitrary source rows with dma_gather (int16 indices,
so the table is split into 4 row-chunks < 2^15 rows; each chunk is exactly
one sub-AllGather region, letting the collectives overlap the producing
compute).

Math (A_hat = D^-1/2 (A+I) D^-1/2): per layer
    out = dinv * (agg of z) [@ W] + b,  z = dinv * (h @ W)
(W folded before aggregation for layers 1/2, after for layer 3), where
    agg_n = z_n + sum_{e: dst=n} z_src.

Device pipeline per core: L0 computes z1^T per 256-node group
(feature-major); each aggregation layer gathers 128-edge tiles from the
table, builds a selection matrix S[e, j] = (dstoff[e] == j) on DVE and
accumulates psum[64, 256] with f32r matmuls (lhsT = gathered rows,
rhs = S); group epilogues run feature-major, and PE transposes convert
back to node-major shard rows for the next table.
"""
import sys
sys.path.insert(0, '/opt/trn_rl_repo')

from contextlib import ExitStack

import numpy as np

from concourse import bass, bacc, tile, mybir, library_config
from concourse.bass_utils import run_bass_kernel_spmd
from concourse.masks import make_identity

# ---- problem constants (hardcoded) ----
N_NODES = 100000
IN_DIM, HID_DIM, OUT_DIM = 128, 64, 7
N_CORES = 8
NS_RAW = N_NODES // N_CORES          # 12500 real nodes per core
P = 128
G = 256                              # nodes per aggregation group
NGRP = 49                            # 12544 / 256
NS = NGRP * G                        # 12544 padded shard size
V = NS * N_CORES                     # 100352 table rows
SUB_GRPS = [13, 12, 12, 12]          # groups per sub-AllGather / idx chunk
N_CHUNK = len(SUB_GRPS)
WAVE_SPLITS = {13: [5, 4, 4], 12: [4, 4, 4]}
NI_MAX = 1024                        # dma_gather idxs per instruction cap

f32 = mybir.dt.float32
f32r = mybir.dt.float32r
i16 = mybir.dt.int16

SUB_G0 = np.cumsum([0] + SUB_GRPS)           # group start per sub
SUB_ROWS = [g * G for g in SUB_GRPS]         # shard rows per sub
SUB_R0 = np.cumsum([0] + SUB_ROWS)           # shard row start per sub
CHUNK_ROWS = [r * N_CORES for r in SUB_ROWS]  # table rows per chunk
CHUNK_BASE = np.cumsum([0] + CHUNK_ROWS)
SUB_OF_GROUP = np.concatenate(
    [np.full(n, s, dtype=np.int64) for s, n in enumerate(SUB_GRPS)])

# waves: list of (sub, [groups])
WAVES = []
for s in range(N_CHUNK):
    g0 = SUB_G0[s]
    for w in WAVE_SPLITS[SUB_GRPS[s]]:
        WAVES.append((s, list(range(g0, g0 + w))))
        g0 += w
WGRP_MAX = max(len(wg) for _, wg in WAVES)


def _host_prep(edge_index):
    """Partition/permute/pad the graph into a static structure shared by all
    cores (cores differ only in input data, not program shape)."""
    src = np.asarray(edge_index[0], dtype=np.int64)
    dst = np.asarray(edge_index[1], dtype=np.int64)
    deg = np.bincount(dst, minlength=N_NODES).astype(np.float64) + 1.0
    dinv = (1.0 / np.sqrt(deg)).astype(np.float32)

    core_of = dst // NS_RAW
    perms = []
    for c in range(N_CORES):
        sel = core_of == c
        dl = (dst[sel] - c * NS_RAW).astype(np.int64)
        cnt = np.bincount(dl, minlength=NS)
        order = np.argsort(-cnt, kind='stable')
        gload = np.zeros(NGRP, dtype=np.int64)
        gfill = np.zeros(NGRP, dtype=np.int64)
        perm = np.empty(NS, dtype=np.int64)
        for node in order:
            cand = np.where(gfill < G)[0]
            gsel = cand[np.argmin(gload[cand])]
            perm[node] = gsel * G + gfill[gsel]
            gfill[gsel] += 1
            gload[gsel] += cnt[node]
        perms.append(perm)

    def trow_chunkidx(nodes):
        """global node id -> (chunk, in-chunk row)"""
        c = nodes // NS_RAW
        loc = np.empty(len(nodes), dtype=np.int64)
        for cc in range(N_CORES):
            m = c == cc
            loc[m] = perms[cc][nodes[m] % NS_RAW]
        g = loc // G
        s = SUB_OF_GROUP[g]
        inrow = c * np.array(SUB_ROWS)[s] + (loc - SUB_R0[s])
        return s, inrow

    src_ch, src_row = trow_chunkidx(src)

    # per-core (group, chunk) runs
    run_lens = np.zeros((N_CORES, NGRP, N_CHUNK), dtype=np.int64)
    edge_lists = []
    for c in range(N_CORES):
        sel = core_of == c
        rows_c, ch_c = src_row[sel], src_ch[sel]
        d_new = perms[c][(dst[sel] - c * NS_RAW)]
        grp, off = d_new // G, d_new % G
        runs = {}
        for g in range(NGRP):
            gm = grp == g
            for k in range(N_CHUNK):
                m = gm & (ch_c == k)
                runs[(g, k)] = (rows_c[m], off[m])
                run_lens[c, g, k] = m.sum()
        edge_lists.append(runs)

    # static tiles per (group, chunk): max over cores
    t_arr = np.ceil(run_lens.max(axis=0) / P).astype(np.int64)  # [NGRP, N_CHUNK]

    # flat tile order + instruction plan (identical for all cores)
    instr_plan = []   # (wave_idx, chunk, tile_off_in_wavechunk, n_idx, idx_col0)
    tiles_meta = []   # flat: (wave_idx, chunk, group, j)
    wave_tiles = []   # tiles per wave
    idx_cols = 0
    for wi, (s, wg) in enumerate(WAVES):
        wt = 0
        for k in range(N_CHUNK):
            slots = int(t_arr[wg, k].sum()) * P
            done = 0
            while done < slots:
                ni = min(NI_MAX, slots - done)
                instr_plan.append((wi, k, done // P, ni, idx_cols))
                idx_cols += ni // 16
                done += ni
            for g in wg:
                for j in range(int(t_arr[g, k])):
                    tiles_meta.append((wi, k, g, j))
                wt += int(t_arr[g, k])
        wave_tiles.append(wt)
    tt = len(tiles_meta)

    # flat slot base of each (wave, chunk) region
    wc_tilebase = {}
    ti = 0
    for wi, (s, wg) in enumerate(WAVES):
        for k in range(N_CHUNK):
            wc_tilebase[(wi, k)] = ti
            ti += int(t_arr[wg, k].sum())

    idx_arrs, doff_arrs = [], []
    for c in range(N_CORES):
        runs = edge_lists[c]
        flat_idx = np.zeros(tt * P, dtype=np.int16)
        flat_off = np.full(tt * P, -1.0, dtype=np.float32)
        pos = 0
        for (wi, k, g, j) in tiles_meta:
            rows, offs = runs[(g, k)]
            a, b = j * P, min((j + 1) * P, len(rows))
            n = max(0, b - a)
            if n > 0:
                flat_idx[pos:pos + n] = rows[a:b].astype(np.int16)
                flat_off[pos:pos + n] = offs[a:b].astype(np.float32)
            pos += P
        idx_wrapped = np.zeros((P, idx_cols), dtype=np.int16)
        for (wi, k, toff, ni, col0) in instr_plan:
            s0 = (wc_tilebase[(wi, k)] + toff) * P
            blk = flat_idx[s0:s0 + ni].reshape(ni // 16, 16).T
            idx_wrapped[:, col0:col0 + ni // 16] = np.tile(blk, (8, 1))
        idx_arrs.append(idx_wrapped)
        doff_arrs.append(flat_off.reshape(tt, P).T.copy())

    return dict(
        dinv=dinv, perms=perms, t_arr=t_arr, instr_plan=instr_plan,
        tiles_meta=tiles_meta, tt=tt, wave_tiles=wave_tiles,
        wc_tilebase=wc_tilebase, idx_arrs=idx_arrs, doff_arrs=doff_arrs,
        idx_cols=idx_cols,
    )


def _build_program(S, repeat=1, ag_mode='collective', skip=()):
    t_arr, instr_plan, tt, idx_cols = (
        S['t_arr'], S['instr_plan'], S['tt'], S['idx_cols'])
    wave_tiles, wc_tilebase = S['wave_tiles'], S['wc_tilebase']
    mw_tiles_max = max(wave_tiles)

    nc = bacc.Bacc("TRN2", target_bir_lowering=False, debug=False,
                   num_devices=N_CORES)

    xT_d = nc.dram_tensor("xT", [P, NS], f32r, kind="ExternalInput")
    dinvrep_d = nc.dram_tensor("dinvrep", [HID_DIM, NS], f32, kind="ExternalInput")
    idx_d = nc.dram_tensor("idx", [P, idx_cols], i16, kind="ExternalInput")
    doff_d = nc.dram_tensor("doff", [P, tt], f32, kind="ExternalInput")
    W1_d = nc.dram_tensor("W1", [IN_DIM, HID_DIM], f32r, kind="ExternalInput")
    W2_d = nc.dram_tensor("W2", [HID_DIM, HID_DIM], f32r, kind="ExternalInput")
    W3_d = nc.dram_tensor("W3", [HID_DIM, 8], f32r, kind="ExternalInput")
    b1_d = nc.dram_tensor("b1", [HID_DIM, 1], f32, kind="ExternalInput")
    b2_d = nc.dram_tensor("b2", [HID_DIM, 1], f32, kind="ExternalInput")
    b3_d = nc.dram_tensor("b3", [8, 1], f32, kind="ExternalInput")
    out_d = nc.dram_tensor("out_shard", [NS, 8], f32, kind="ExternalOutput")

    nc.gpsimd.load_library(library_config.mlp)

    with tile.TileContext(nc) as tc:
        stack = ExitStack()
        zsh = [tc.tile([NS, HID_DIM], f32r, space="DRAM", name=f"zsh{i}")[0]
               for i in range(3)]
        dramp = stack.enter_context(
            tc.tile_pool(name="dramp", bufs=1, space="DRAM"))
        def alloc_tables(rep):
            return [[dramp.tile([CHUNK_ROWS[k], HID_DIM], f32r,
                                addr_space="Shared",
                                name=f"table{rep}_{i}_{k}",
                                tag=f"table{rep}_{i}_{k}")
                     for k in range(N_CHUNK)] for i in range(3)]
        const = stack.enter_context(tc.tile_pool(name="const", bufs=1))

        R_i = const.tile([P, G], mybir.dt.int32)
        nc.gpsimd.iota(R_i[:], pattern=[[1, G]], base=0, channel_multiplier=0)
        R_f = const.tile([P, G], f32)
        nc.vector.tensor_copy(out=R_f[:], in_=R_i[:])
        ident = const.tile([P, P], f32)
        make_identity(nc, ident[:])
        ident_r = const.tile([P, P], f32r)
        nc.vector.tensor_copy(out=ident_r[:], in_=ident[:])

        St_dummy = None
        if 'sbuild' in skip:
            St_dummy = const.tile([P, G], f32r)
            nc.vector.tensor_copy(out=St_dummy[:], in_=R_f[:])
        W1_t = const.tile([IN_DIM, HID_DIM], f32r)
        nc.sync.dma_start(out=W1_t[:], in_=W1_d[:])
        W2_t = const.tile([HID_DIM, HID_DIM], f32r)
        nc.sync.dma_start(out=W2_t[:], in_=W2_d[:])
        W3_t = const.tile([HID_DIM, 8], f32r)
        nc.sync.dma_start(out=W3_t[:], in_=W3_d[:])
        b1_t = const.tile([HID_DIM, 1], f32)
        nc.sync.dma_start(out=b1_t[:], in_=b1_d[:])
        b2_t = const.tile([HID_DIM, 1], f32)
        nc.sync.dma_start(out=b2_t[:], in_=b2_d[:])
        b3_t = const.tile([8, 1], f32)
        nc.sync.dma_start(out=b3_t[:], in_=b3_d[:])
        idx_t = const.tile([P, idx_cols], i16)
        nc.sync.dma_start(out=idx_t[:], in_=idx_d[:])
        doff_t = const.tile([P, tt], f32)
        nc.sync.dma_start(out=doff_t[:], in_=doff_d[:])

        zTd = [tc.tile([HID_DIM, NS], f32r, space="DRAM", name=f"zTd{i}")[0]
               for i in range(2)]

        sbuf = stack.enter_context(tc.tile_pool(name="sbuf", bufs=3))
        spool = stack.enter_context(tc.tile_pool(name="spool", bufs=6))
        wavep = stack.enter_context(tc.tile_pool(name="wavep", bufs=2))
        znodep = stack.enter_context(tc.tile_pool(name="znodep", bufs=2))
        psum_agg = stack.enter_context(
            tc.tile_pool(name="psum_agg", bufs=3, space="PSUM"))
        psum_mm2 = stack.enter_context(
            tc.tile_pool(name="psum_mm2", bufs=2, space="PSUM"))
        psum_tr = stack.enter_context(
            tc.tile_pool(name="psum_tr", bufs=2, space="PSUM"))

        def load_dvw(wg):
            w0, wn = wg[0] * G, len(wg) * G
            dvw = wavep.tile([HID_DIM, wn], f32, tag="dvw",
                             padded_shape=[HID_DIM, WGRP_MAX * G])
            nc.sync.dma_start(out=dvw[:], in_=dinvrep_d[:, w0:w0 + wn])
            return dvw

        def store_wave_fm(zcw, wg, fdim, node_dram, zT_target):
            """Batch-transpose the feature-major wave tile [fdim, wn] into
            node-major [wn, fdim] rows of node_dram; also stash feature-major
            into zT_target if given."""
            w0, wn = wg[0] * G, len(wg) * G
            nch = wn // P
            for blk0 in range(0, nch, 8):
                nb = min(8, nch - blk0)
                ptr = psum_tr.tile([P, nb * fdim], f32r, tag="ptr",
                                   padded_shape=[P, 8 * HID_DIM])
                for i in range(nb):
                    nc.tensor.transpose(
                        out=ptr[:, i * fdim:(i + 1) * fdim],
                        in_=zcw[:fdim, (blk0 + i) * P:(blk0 + i + 1) * P],
                        identity=ident_r[:fdim, :fdim])
                zn = znodep.tile([P, nb * fdim], f32r, tag="zn",
                                 padded_shape=[P, 8 * HID_DIM])
                nc.vector.tensor_copy(out=zn[:], in_=ptr[:])
                dst = node_dram[w0 + blk0 * P: w0 + (blk0 + nb) * P, :]
                src_ap = zn[:] if node_dram is not out_d else zn[:].bitcast(f32)
                nc.scalar.dma_start(
                    out=dst.rearrange("(c p) f -> p c f", p=P),
                    in_=src_ap.rearrange("p (c f) -> p c f", f=fdim))
            if zT_target is not None:
                nc.scalar.dma_start(out=zT_target[:, w0:w0 + wn], in_=zcw[:])

        def sub_allgather(zsh_t, table_t, s):
            r0, rn = SUB_R0[s], SUB_ROWS[s]
            if ag_mode == 'local':
                nc.scalar.dma_start(out=table_t[s][0:rn, :],
                                    in_=zsh_t[r0:r0 + rn, :])
                return
            nc.gpsimd.collective_compute(
                "AllGather", mybir.AluOpType.bypass,
                replica_groups=[list(range(N_CORES))],
                ins=[zsh_t[r0:r0 + rn, :]],
                outs=[table_t[s][:]])

        for _rep in range(repeat):
            tables = alloc_tables(_rep)
            # ---------- L0: z1 = dinv .* (x @ W1), feature-major ----------
            for wi, (s, wg) in enumerate(WAVES):
                w0, wn = wg[0] * G, len(wg) * G
                xw = wavep.tile([P, wn], f32r, tag="xw",
                                padded_shape=[P, WGRP_MAX * G])
                nc.sync.dma_start(out=xw[:], in_=xT_d[:, w0:w0 + wn])
                dvw = load_dvw(wg)
                zcw = wavep.tile([HID_DIM, wn], f32r, tag="zcw",
                                 padded_shape=[HID_DIM, WGRP_MAX * G])
                for g in wg:
                    c0 = (g - wg[0]) * G
                    ps = psum_agg.tile([HID_DIM, G], f32, tag="ps")
                    nc.tensor.matmul(out=ps[:], lhsT=W1_t[:],
                                     rhs=xw[:, c0:c0 + G],
                                     start=True, stop=True)
                    nc.vector.tensor_tensor(
                        out=zcw[:, c0:c0 + G], in0=ps[:],
                        in1=dvw[:, c0:c0 + G], op=mybir.AluOpType.mult)
                store_wave_fm(zcw, wg, HID_DIM, zsh[0], zTd[0])
                if wi + 1 == len(WAVES) or WAVES[wi + 1][0] != s:
                    sub_allgather(zsh[0], tables[0], s)

            # ---------- aggregation layers ----------
            def agg_layer(layer, table, zT_in, zT_out_d, W_next, bias_t,
                          final=False):
                for wi, (s, wg) in enumerate(WAVES):
                    w0, wn = wg[0] * G, len(wg) * G
                    wtiles = wave_tiles[wi]
                    mw = wavep.tile([P, wtiles, HID_DIM], f32r, tag="mw",
                                    padded_shape=[P, mw_tiles_max, HID_DIM])
                    wave_t0 = wc_tilebase[(wi, 0)]
                    if 'gather' in skip:
                        nc.scalar.memzero(mw[:])
                    for (wi2, k, toff, ni, col0) in instr_plan:
                        if wi2 != wi or 'gather' in skip:
                            continue
                        ck = wc_tilebase[(wi, k)] - wave_t0
                        nc.gpsimd.dma_gather(
                            out_ap=mw[:, ck + toff: ck + toff + ni // P, :],
                            in_ap=table[k][:],
                            idxs_ap=idx_t[:, col0: col0 + ni // 16],
                            num_idxs=ni, num_idxs_reg=ni, elem_size=HID_DIM,
                            single_packet=True,
                        )
                    zsw = wavep.tile([HID_DIM, wn], f32r, tag="zsw",
                                     padded_shape=[HID_DIM, WGRP_MAX * G])
                    nc.sync.dma_start(out=zsw[:], in_=zT_in[:, w0:w0 + wn])
                    dvw = load_dvw(wg)
                    if final:
                        zcw = wavep.tile([8, wn], f32r, tag="ocw",
                                         padded_shape=[8, WGRP_MAX * G])
                    else:
                        zcw = wavep.tile([HID_DIM, wn], f32r, tag="zcw",
                                         padded_shape=[HID_DIM, WGRP_MAX * G])
                    for gi, g in enumerate(wg):
                        ps = psum_agg.tile([HID_DIM, G], f32, tag="ps")
                        n_mm = int(t_arr[g].sum())
                        if 'aggmm' in skip:
                            nc.tensor.matmul(
                                out=ps[:], lhsT=mw[:, 0, :],
                                rhs=St_dummy[:] if St_dummy is not None
                                else R_f[:].bitcast(f32r),
                                start=True, stop=True)
                        mm_i = 0
                        for k in range(N_CHUNK):
                            ck = wc_tilebase[(wi, k)] - wave_t0
                            jbase = int(t_arr[wg[0]:g, k].sum())
                            for j in range(int(t_arr[g, k])):
                                wt = ck + jbase + j
                                ft = wave_t0 + wt if k == 0 else (
                                    wc_tilebase[(wi, k)] + jbase + j)
                                if 'sbuild' in skip:
                                    St = St_dummy
                                else:
                                    St = spool.tile([P, G], f32r, tag="St")
                                    nc.vector.tensor_scalar(
                                        out=St[:], in0=R_f[:],
                                        scalar1=doff_t[:, ft:ft + 1],
                                        scalar2=None,
                                        op0=mybir.AluOpType.is_equal)
                                if 'aggmm' not in skip:
                                    nc.tensor.matmul(
                                        out=ps[:], lhsT=mw[:, wt, :], rhs=St[:],
                                        start=(mm_i == 0),
                                        stop=(mm_i == n_mm - 1))
                                mm_i += 1
                        # ---- epilogue for group g ----
                        c0 = (g - wg[0]) * G
                        c1 = c0 + G
                        e1 = sbuf.tile([HID_DIM, G], f32, tag="e1")
                        nc.vector.tensor_tensor(out=e1[:], in0=ps[:],
                                                in1=zsw[:, c0:c1],
                                                op=mybir.AluOpType.add)
                        if final:
                            e2 = sbuf.tile([HID_DIM, G], f32r, tag="e2")
                            nc.vector.tensor_tensor(out=e2[:], in0=e1[:],
                                                    in1=dvw[:, c0:c1],
                                                    op=mybir.AluOpType.mult)
                            po = psum_mm2.tile([8, G], f32, tag="po")
                            nc.tensor.matmul(out=po[:], lhsT=W3_t[:],
                                             rhs=e2[:], start=True, stop=True)
                            nc.vector.tensor_scalar(
                                out=zcw[:, c0:c1], in0=po[:],
                                scalar1=b3_t[:, :1],
                                scalar2=None, op0=mybir.AluOpType.add)
                        else:
                            e2 = sbuf.tile([HID_DIM, G], f32, tag="e2")
                            nc.vector.tensor_tensor(out=e2[:], in0=e1[:],
                                                    in1=dvw[:, c0:c1],
                                                    op=mybir.AluOpType.mult)
                            hT = sbuf.tile([HID_DIM, G], f32r, tag="hT")
                            nc.vector.tensor_scalar(
                                out=hT[:], in0=e2[:], scalar1=bias_t[:, :1],
                                scalar2=0.0, op0=mybir.AluOpType.add,
                                op1=mybir.AluOpType.max)
                            if W_next is not None:
                                po = psum_mm2.tile([HID_DIM, G], f32, tag="po")
                                nc.tensor.matmul(out=po[:], lhsT=W_next[:],
                                                 rhs=hT[:], start=True,
                                                 stop=True)
                                nc.vector.tensor_tensor(
                                    out=zcw[:, c0:c1], in0=po[:],
                                    in1=dvw[:, c0:c1],
                                    op=mybir.AluOpType.mult)
                            else:
                                nc.vector.tensor_tensor(
                                    out=zcw[:, c0:c1], in0=hT[:],
                                    in1=dvw[:, c0:c1],
                                    op=mybir.AluOpType.mult)
                    if final:
                        store_wave_fm(zcw, wg, 8, out_d, None)
                    else:
                        store_wave_fm(zcw, wg, HID_DIM, zsh[layer],
                                      zT_out_d)
                        if wi + 1 == len(WAVES) or WAVES[wi + 1][0] != s:
                            sub_allgather(zsh[layer], tables[layer], s)

            agg_layer(1, tables[0], zTd[0], zTd[1], W2_t, b1_t)
            agg_layer(2, tables[1], zTd[1], zTd[0], None, b2_t)
            agg_layer(3, tables[2], zTd[0], None, None, b3_t, final=True)
        stack.close()

    nc.finalize()
    return nc


def _make_in_maps(S, x, W1, b1, W2, b2, W3, b3):
    dinv = S['dinv']
    W3p = np.zeros((HID_DIM, 8), np.float32)
    W3p[:, :OUT_DIM] = W3
    b3p = np.zeros((8, 1), np.float32)
    b3p[:OUT_DIM, 0] = b3
    in_maps = []
    for c in range(N_CORES):
        perm = S['perms'][c]
        xs = np.zeros((NS, IN_DIM), np.float32)
        dv = np.ones(NS, np.float32)
        xs[perm[:NS_RAW]] = x[c * NS_RAW:(c + 1) * NS_RAW]
        dv[perm[:NS_RAW]] = dinv[c * NS_RAW:(c + 1) * NS_RAW]
        in_maps.append({
            "xT": np.ascontiguousarray(xs.T),
            "dinvrep": np.ascontiguousarray(
                np.broadcast_to(dv[None, :], (HID_DIM, NS))),
            "idx": S['idx_arrs'][c],
            "doff": S['doff_arrs'][c],
            "W1": W1, "W2": W2, "W3": W3p,
            "b1": b1.reshape(-1, 1), "b2": b2.reshape(-1, 1), "b3": b3p,
        })
    return in_maps


_LAST = {}


def kernel(x, edge_index, W1, b1, W2, b2, W3, b3):
    x = np.asarray(x, dtype=np.float32)
    W1 = np.asarray(W1, dtype=np.float32)
    W2 = np.asarray(W2, dtype=np.float32)
    W3 = np.asarray(W3, dtype=np.float32)
    b1 = np.asarray(b1, dtype=np.float32)
    b2 = np.asarray(b2, dtype=np.float32)
    b3 = np.asarray(b3, dtype=np.float32)

    S = _host_prep(edge_index)
    nc = _build_program(S)
    in_maps = _make_in_maps(S, x, W1, b1, W2, b2, W3, b3)

    res = run_bass_kernel_spmd(nc, in_maps, core_ids=list(range(N_CORES)))

    _LAST['S'] = S
    _LAST['in_maps'] = in_maps

    out = np.empty((N_NODES, OUT_DIM), np.float32)
    for c in range(N_CORES):
        shard = res.results[c]["out_shard"]       # [NS, 8]
        perm = S['perms'][c]
        out[c * NS_RAW:(c + 1) * NS_RAW] = shard[perm[:NS_RAW], :OUT_DIM]
    return out


def measure_exec_ns(repeats=(1, 5), iters=6, ag_mode='collective', skip=()):
    """Estimate HW exec time by building R-times-repeated variants of the
    full pipeline and differencing pipelined wall-clock."""
    import time
    import jax
    from jax.sharding import Mesh, PartitionSpec, NamedSharding
    from jax.experimental.shard_map import shard_map
    from concourse import bass2jax
    from concourse.bass2jax import _bass_exec_p, install_neuronx_cc_hook

    S, in_maps = _LAST['S'], _LAST['in_maps']
    install_neuronx_cc_hook()
    per_call = {}
    for R in repeats:
        nc = _build_program(S, repeat=R, ag_mode=ag_mode, skip=skip)
        partition_name = (nc.partition_id_tensor.name
                          if nc.partition_id_tensor else None)
        in_names, out_names, out_avals, zero_outs = [], [], [], []
        for alloc in nc.m.functions[0].allocations:
            if not isinstance(alloc, mybir.MemoryLocationSet):
                continue
            name = alloc.memorylocations[0].name
            if alloc.kind == "ExternalInput":
                if name != partition_name:
                    in_names.append(name)
            elif alloc.kind == "ExternalOutput":
                out_names.append(name)
                shape = tuple(alloc.tensor_shape)
                dtype = mybir.dt.np(alloc.dtype)
                out_avals.append(jax.core.ShapedArray(shape, dtype))
                zero_outs.append(np.zeros(shape, dtype))
        all_in = list(in_names) + list(out_names)
        if partition_name:
            all_in.append(partition_name)

        def _body(*args, _nc=nc, _avals=tuple(out_avals), _in=tuple(all_in),
                  _out=tuple(out_names)):
            operands = list(args)
            operands.append(bass2jax.partition_id_tensor())
            return tuple(_bass_exec_p.bind(
                *operands, out_avals=_avals, in_names=_in, out_names=_out,
                lowering_input_output_aliases=(), sim_require_finite=True,
                sim_require_nnan=True, nc=_nc))

        devices = jax.devices()[:N_CORES]
        mesh = Mesh(np.asarray(devices), ("core",))
        nsp = len(in_names) + len(zero_outs)
        sharded = jax.jit(shard_map(
            _body, mesh=mesh, in_specs=(PartitionSpec("core"),) * nsp,
            out_specs=(PartitionSpec("core"),) * len(out_names),
            check_rep=False), keep_unused=True)
        args = [np.concatenate([np.asarray(in_maps[c][n]) for c in
                                range(N_CORES)], axis=0) for n in in_names]
        args += [np.zeros((N_CORES * z.shape[0], *z.shape[1:]), z.dtype)
                 for z in zero_outs]
        sh = NamedSharding(mesh, PartitionSpec("core"))
        args = [jax.device_put(a, sh) for a in args]
        outs = sharded(*args)
        jax.block_until_ready(outs)
        best = None
        for _ in range(iters):
            t0 = time.perf_counter()
            got = [sharded(*args) for _ in range(4)]
            jax.block_until_ready(got)
            dt = (time.perf_counter() - t0) / 4
            best = dt if best is None else min(best, dt)
        per_call[R] = best
    r0, r1 = repeats
    est = (per_call[r1] - per_call[r0]) / (r1 - r0)
    return max(1, int(est * 1e9))



# revision 11
# speedup vs baseline: 4.0476x; 2.9457x over previous
"""GCN (3-layer, PyG GCNConv semantics) on 8 Trainium2 NeuronCores.

Sharding: nodes are partitioned across the 8 cores by destination id
(graph-parallel). Each core aggregates messages for its own node shard; the
per-layer node features ("tables") are replicated via chunked AllGathers so
every core can gather arbitrary source rows with dma_gather (int16 indices,
so the table is split into 4 row-chunks < 2^15 rows; each chunk is exactly
one sub-AllGather region, letting the collectives overlap the producing
compute).

Math (A_hat = D^-1/2 (A+I) D^-1/2): per layer
    out = dinv * (agg of z) [@ W] + b,  z = dinv * (h @ W)
(W folded before aggregation for layers 1/2, after for layer 3), where
    agg_n = z_n + sum_{e: dst=n} z_src.

Device pipeline per core: L0 computes z1^T per 256-node group
(feature-major); each aggregation layer gathers 128-edge tiles from the
table, builds a selection matrix S[e, j] = (dstoff[e] == j) on DVE and
accumulates psum[64, 256] with f32r matmuls (lhsT = gathered rows,
rhs = S); group epilogues run feature-major, and PE transposes convert
back to node-major shard rows for the next table.
"""
import sys
sys.path.insert(0, '/opt/trn_rl_repo')

from contextlib import ExitStack

import numpy as np

from concourse import bass, bacc, tile, mybir, library_config
from concourse.bass_utils import run_bass_kernel_spmd
from concourse.masks import make_identity

# ---- problem constants (hardcoded) ----
N_NODES = 100000
IN_DIM, HID_DIM, OUT_DIM = 128, 64, 7
N_CORES = 8
NS_RAW = N_NODES // N_CORES          # 12500 real nodes per core
P = 128
G = 256                              # nodes per aggregation group
NGRP = 49                            # 12544 / 256
NS = NGRP * G                        # 12544 padded shard size
V = NS * N_CORES                     # 100352 table rows
SUB_GRPS = [13, 12, 12, 12]          # groups per sub-AllGather / idx chunk
N_CHUNK = len(SUB_GRPS)
WAVE_SPLITS = {13: [5, 4, 4], 12: [4, 4, 4]}
NI_MAX = 1024                        # dma_gather idxs per instruction cap

f32 = mybir.dt.float32
f32r = mybir.dt.float32r
i16 = mybir.dt.int16

SUB_G0 = np.cumsum([0] + SUB_GRPS)           # group start per sub
SUB_ROWS = [g * G for g in SUB_GRPS]         # shard rows per sub
SUB_R0 = np.cumsum([0] + SUB_ROWS)           # shard row start per sub
CHUNK_ROWS = [r * N_CORES for r in SUB_ROWS]  # table rows per chunk
CHUNK_BASE = np.cumsum([0] + CHUNK_ROWS)
SUB_OF_GROUP = np.concatenate(
    [np.full(n, s, dtype=np.int64) for s, n in enumerate(SUB_GRPS)])

# waves: list of (sub, [groups])
WAVES = []
for s in range(N_CHUNK):
    g0 = SUB_G0[s]
    for w in WAVE_SPLITS[SUB_GRPS[s]]:
        WAVES.append((s, list(range(g0, g0 + w))))
        g0 += w
WGRP_MAX = max(len(wg) for _, wg in WAVES)


def _host_prep(edge_index):
    """Partition/permute/pad the graph into a static structure shared by all
    cores (cores differ only in input data, not program shape)."""
    src = np.asarray(edge_index[0], dtype=np.int64)
    dst = np.asarray(edge_index[1], dtype=np.int64)
    deg = np.bincount(dst, minlength=N_NODES).astype(np.float64) + 1.0
    dinv = (1.0 / np.sqrt(deg)).astype(np.float32)

    core_of = dst // NS_RAW
    perms = []
    for c in range(N_CORES):
        sel = core_of == c
        dl = (dst[sel] - c * NS_RAW).astype(np.int64)
        cnt = np.bincount(dl, minlength=NS)
        order = np.argsort(-cnt, kind='stable')
        gload = np.zeros(NGRP, dtype=np.int64)
        gfill = np.zeros(NGRP, dtype=np.int64)
        perm = np.empty(NS, dtype=np.int64)
        for node in order:
            cand = np.where(gfill < G)[0]
            gsel = cand[np.argmin(gload[cand])]
            perm[node] = gsel * G + gfill[gsel]
            gfill[gsel] += 1
            gload[gsel] += cnt[node]
        perms.append(perm)

    def trow_chunkidx(nodes):
        """global node id -> (chunk, in-chunk row)"""
        c = nodes // NS_RAW
        loc = np.empty(len(nodes), dtype=np.int64)
        for cc in range(N_CORES):
            m = c == cc
            loc[m] = perms[cc][nodes[m] % NS_RAW]
        g = loc // G
        s = SUB_OF_GROUP[g]
        inrow = c * np.array(SUB_ROWS)[s] + (loc - SUB_R0[s])
        return s, inrow

    src_ch, src_row = trow_chunkidx(src)

    # per-core (group, chunk) runs
    run_lens = np.zeros((N_CORES, NGRP, N_CHUNK), dtype=np.int64)
    edge_lists = []
    for c in range(N_CORES):
        sel = core_of == c
        rows_c, ch_c = src_row[sel], src_ch[sel]
        d_new = perms[c][(dst[sel] - c * NS_RAW)]
        grp, off = d_new // G, d_new % G
        runs = {}
        for g in range(NGRP):
            gm = grp == g
            for k in range(N_CHUNK):
                m = gm & (ch_c == k)
                runs[(g, k)] = (rows_c[m], off[m])
                run_lens[c, g, k] = m.sum()
        edge_lists.append(runs)

    # static tiles per (group, chunk): max over cores
    t_arr = np.ceil(run_lens.max(axis=0) / P).astype(np.int64)  # [NGRP, N_CHUNK]

    # flat tile order + instruction plan (identical for all cores)
    instr_plan = []   # (wave_idx, chunk, tile_off_in_wavechunk, n_idx, idx_col0)
    tiles_meta = []   # flat: (wave_idx, chunk, group, j)
    wave_tiles = []   # tiles per wave
    idx_cols = 0
    for wi, (s, wg) in enumerate(WAVES):
        wt = 0
        for k in range(N_CHUNK):
            slots = int(t_arr[wg, k].sum()) * P
            done = 0
            while done < slots:
                ni = min(NI_MAX, slots - done)
                instr_plan.append((wi, k, done // P, ni, idx_cols))
                idx_cols += ni // 16
                done += ni
            for g in wg:
                for j in range(int(t_arr[g, k])):
                    tiles_meta.append((wi, k, g, j))
                wt += int(t_arr[g, k])
        wave_tiles.append(wt)
    tt = len(tiles_meta)

    # flat slot base of each (wave, chunk) region
    wc_tilebase = {}
    ti = 0
    for wi, (s, wg) in enumerate(WAVES):
        for k in range(N_CHUNK):
            wc_tilebase[(wi, k)] = ti
            ti += int(t_arr[wg, k].sum())

    idx_arrs, doff_arrs = [], []
    for c in range(N_CORES):
        runs = edge_lists[c]
        flat_idx = np.zeros(tt * P, dtype=np.int16)
        flat_off = np.full(tt * P, -1.0, dtype=np.float32)
        pos = 0
        for (wi, k, g, j) in tiles_meta:
            rows, offs = runs[(g, k)]
            a, b = j * P, min((j + 1) * P, len(rows))
            n = max(0, b - a)
            if n > 0:
                flat_idx[pos:pos + n] = rows[a:b].astype(np.int16)
                flat_off[pos:pos + n] = offs[a:b].astype(np.float32)
            pos += P
        idx_wrapped = np.zeros((P, idx_cols), dtype=np.int16)
        for (wi, k, toff, ni, col0) in instr_plan:
            s0 = (wc_tilebase[(wi, k)] + toff) * P
            blk = flat_idx[s0:s0 + ni].reshape(ni // 16, 16).T
            idx_wrapped[:, col0:col0 + ni // 16] = np.tile(blk, (8, 1))
        idx_arrs.append(idx_wrapped)
        doff_arrs.append(flat_off.reshape(tt, P).T.copy())

    return dict(
        dinv=dinv, perms=perms, t_arr=t_arr, instr_plan=instr_plan,
        tiles_meta=tiles_meta, tt=tt, wave_tiles=wave_tiles,
        wc_tilebase=wc_tilebase, idx_arrs=idx_arrs, doff_arrs=doff_arrs,
        idx_cols=idx_cols,
    )


def _build_program(S, repeat=1, ag_mode='collective', skip=()):
    t_arr, instr_plan, tt, idx_cols = (
        S['t_arr'], S['instr_plan'], S['tt'], S['idx_cols'])
    wave_tiles, wc_tilebase = S['wave_tiles'], S['wc_tilebase']
    mw_tiles_max = max(wave_tiles)

    nc = bacc.Bacc("TRN2", target_bir_lowering=False, debug=False,
                   num_devices=N_CORES)

    xT_d = nc.dram_tensor("xT", [P, NS], f32r, kind="ExternalInput")
    dinvrep_d = nc.dram_tensor("dinvrep", [HID_DIM, NS], f32, kind="ExternalInput")
    idx_d = nc.dram_tensor("idx", [P, idx_cols], i16, kind="ExternalInput")
    doff_d = nc.dram_tensor("doff", [P, tt], f32, kind="ExternalInput")
    W1_d = nc.dram_tensor("W1", [IN_DIM, HID_DIM], f32r, kind="ExternalInput")
    W2_d = nc.dram_tensor("W2", [HID_DIM, HID_DIM], f32r, kind="ExternalInput")
    W3_d = nc.dram_tensor("W3", [HID_DIM, 8], f32r, kind="ExternalInput")
    b1_d = nc.dram_tensor("b1", [HID_DIM, 1], f32, kind="ExternalInput")
    b2_d = nc.dram_tensor("b2", [HID_DIM, 1], f32, kind="ExternalInput")
    b3_d = nc.dram_tensor("b3", [8, 1], f32, kind="ExternalInput")
    out_d = nc.dram_tensor("out_shard", [NS, 8], f32, kind="ExternalOutput")

    nc.gpsimd.load_library(library_config.mlp)

    with tile.TileContext(nc) as tc:
        stack = ExitStack()
        zsh = [tc.tile([NS, HID_DIM], f32r, space="DRAM", name=f"zsh{i}")[0]
               for i in range(3)]
        dramp = stack.enter_context(
            tc.tile_pool(name="dramp", bufs=1, space="DRAM"))
        def alloc_tables(rep):
            return [[dramp.tile([CHUNK_ROWS[k], HID_DIM], f32r,
                                addr_space="Shared",
                                name=f"table{rep}_{i}_{k}",
                                tag=f"table{rep}_{i}_{k}")
                     for k in range(N_CHUNK)] for i in range(3)]
        const = stack.enter_context(tc.tile_pool(name="const", bufs=1))

        R_i = const.tile([P, G], mybir.dt.int32)
        nc.gpsimd.iota(R_i[:], pattern=[[1, G]], base=0, channel_multiplier=0)
        R_f = const.tile([P, G], f32)
        nc.vector.tensor_copy(out=R_f[:], in_=R_i[:])
        ident = const.tile([P, P], f32)
        make_identity(nc, ident[:])
        ident_r = const.tile([P, P], f32r)
        nc.vector.tensor_copy(out=ident_r[:], in_=ident[:])

        St_dummy = None
        if 'sbuild' in skip:
            St_dummy = const.tile([P, G], f32r)
            nc.vector.tensor_copy(out=St_dummy[:], in_=R_f[:])
        W1_t = const.tile([IN_DIM, HID_DIM], f32r)
        nc.sync.dma_start(out=W1_t[:], in_=W1_d[:])
        W2_t = const.tile([HID_DIM, HID_DIM], f32r)
        nc.sync.dma_start(out=W2_t[:], in_=W2_d[:])
        W3_t = const.tile([HID_DIM, 8], f32r)
        nc.sync.dma_start(out=W3_t[:], in_=W3_d[:])
        b1_t = const.tile([HID_DIM, 1], f32)
        nc.sync.dma_start(out=b1_t[:], in_=b1_d[:])
        b2_t = const.tile([HID_DIM, 1], f32)
        nc.sync.dma_start(out=b2_t[:], in_=b2_d[:])
        b3_t = const.tile([8, 1], f32)
        nc.sync.dma_start(out=b3_t[:], in_=b3_d[:])
        idx_t = const.tile([P, idx_cols], i16)
        nc.sync.dma_start(out=idx_t[:], in_=idx_d[:])
        doff_t = const.tile([P, tt], f32)
        nc.sync.dma_start(out=doff_t[:], in_=doff_d[:])

        zTd = [tc.tile([HID_DIM, NS], f32r, space="DRAM", name=f"zTd{i}")[0]
               for i in range(2)]

        sbuf = stack.enter_context(tc.tile_pool(name="sbuf", bufs=3))
        spool = stack.enter_context(tc.tile_pool(name="spool", bufs=6))
        wavep = stack.enter_context(tc.tile_pool(name="wavep", bufs=2))
        znodep = stack.enter_context(tc.tile_pool(name="znodep", bufs=2))
        psum_agg = stack.enter_context(
            tc.tile_pool(name="psum_agg", bufs=3, space="PSUM"))
        psum_mm2 = stack.enter_context(
            tc.tile_pool(name="psum_mm2", bufs=2, space="PSUM"))
        psum_tr = stack.enter_context(
            tc.tile_pool(name="psum_tr", bufs=2, space="PSUM"))

        def load_dvw(wg):
            w0, wn = wg[0] * G, len(wg) * G
            dvw = wavep.tile([HID_DIM, wn], f32, tag="dvw",
                             padded_shape=[HID_DIM, WGRP_MAX * G])
            nc.sync.dma_start(out=dvw[:], in_=dinvrep_d[:, w0:w0 + wn])
            return dvw

        def store_wave_fm(zcw, wg, fdim, node_dram, zT_target):
            """Batch-transpose the feature-major wave tile [fdim, wn] into
            node-major [wn, fdim] rows of node_dram; also stash feature-major
            into zT_target if given."""
            w0, wn = wg[0] * G, len(wg) * G
            nch = wn // P
            for blk0 in range(0, nch, 8):
                nb = min(8, nch - blk0)
                ptr = psum_tr.tile([P, nb * fdim], f32r, tag="ptr",
                                   padded_shape=[P, 8 * HID_DIM])
                for i in range(nb):
                    nc.tensor.transpose(
                        out=ptr[:, i * fdim:(i + 1) * fdim],
                        in_=zcw[:fdim, (blk0 + i) * P:(blk0 + i + 1) * P],
                        identity=ident_r[:fdim, :fdim])
                zn = znodep.tile([P, nb * fdim], f32r, tag="zn",
                                 padded_shape=[P, 8 * HID_DIM])
                nc.vector.tensor_copy(out=zn[:], in_=ptr[:])
                dst = node_dram[w0 + blk0 * P: w0 + (blk0 + nb) * P, :]
                src_ap = zn[:] if node_dram is not out_d else zn[:].bitcast(f32)
                nc.scalar.dma_start(
                    out=dst.rearrange("(c p) f -> p c f", p=P),
                    in_=src_ap.rearrange("p (c f) -> p c f", f=fdim))
            if zT_target is not None:
                nc.scalar.dma_start(out=zT_target[:, w0:w0 + wn], in_=zcw[:])

        def sub_allgather(zsh_t, table_t, s):
            r0, rn = SUB_R0[s], SUB_ROWS[s]
            if ag_mode == 'local':
                nc.scalar.dma_start(out=table_t[s][0:rn, :],
                                    in_=zsh_t[r0:r0 + rn, :])
                return
            nc.gpsimd.collective_compute(
                "AllGather", mybir.AluOpType.bypass,
                replica_groups=[list(range(N_CORES))],
                ins=[zsh_t[r0:r0 + rn, :]],
                outs=[table_t[s][:]])

        for _rep in range(repeat):
            tables = alloc_tables(_rep)
            # ---------- L0: z1 = dinv .* (x @ W1), feature-major ----------
            for wi, (s, wg) in enumerate(WAVES):
                w0, wn = wg[0] * G, len(wg) * G
                xw = wavep.tile([P, wn], f32r, tag="xw",
                                padded_shape=[P, WGRP_MAX * G])
                nc.sync.dma_start(out=xw[:], in_=xT_d[:, w0:w0 + wn])
                dvw = load_dvw(wg)
                zcw = wavep.tile([HID_DIM, wn], f32r, tag="zcw",
                                 padded_shape=[HID_DIM, WGRP_MAX * G])
                for g in wg:
                    c0 = (g - wg[0]) * G
                    ps = psum_agg.tile([HID_DIM, G], f32, tag="ps")
                    nc.tensor.matmul(out=ps[:], lhsT=W1_t[:],
                                     rhs=xw[:, c0:c0 + G],
                                     start=True, stop=True)
                    nc.vector.tensor_tensor(
                        out=zcw[:, c0:c0 + G], in0=ps[:],
                        in1=dvw[:, c0:c0 + G], op=mybir.AluOpType.mult)
                store_wave_fm(zcw, wg, HID_DIM, zsh[0], zTd[0])
                if wi + 1 == len(WAVES) or WAVES[wi + 1][0] != s:
                    sub_allgather(zsh[0], tables[0], s)

            # ---------- aggregation layers ----------
            def agg_layer(layer, table, zT_in, zT_out_d, W_next, bias_t,
                          final=False):
                for wi, (s, wg) in enumerate(WAVES):
                    w0, wn = wg[0] * G, len(wg) * G
                    wtiles = wave_tiles[wi]
                    mw = wavep.tile([P, wtiles, HID_DIM], f32r, tag="mw",
                                    padded_shape=[P, mw_tiles_max, HID_DIM])
                    wave_t0 = wc_tilebase[(wi, 0)]
                    if 'gather' in skip:
                        nc.vector.tensor_copy(
                            out=mw[:, 0, :],
                            in_=R_f[:, :HID_DIM].bitcast(f32r))
                    for (wi2, k, toff, ni, col0) in instr_plan:
                        if wi2 != wi or 'gather' in skip:
                            continue
                        ck = wc_tilebase[(wi, k)] - wave_t0
                        nc.gpsimd.dma_gather(
                            out_ap=mw[:, ck + toff: ck + toff + ni // P, :],
                            in_ap=table[k][:],
                            idxs_ap=idx_t[:, col0: col0 + ni // 16],
                            num_idxs=ni, num_idxs_reg=ni, elem_size=HID_DIM,
                            single_packet=True,
                        )
                    zsw = wavep.tile([HID_DIM, wn], f32r, tag="zsw",
                                     padded_shape=[HID_DIM, WGRP_MAX * G])
                    nc.sync.dma_start(out=zsw[:], in_=zT_in[:, w0:w0 + wn])
                    dvw = load_dvw(wg)
                    if final:
                        zcw = wavep.tile([8, wn], f32r, tag="ocw",
                                         padded_shape=[8, WGRP_MAX * G])
                    else:
                        zcw = wavep.tile([HID_DIM, wn], f32r, tag="zcw",
                                         padded_shape=[HID_DIM, WGRP_MAX * G])
                    for gi, g in enumerate(wg):
                        ps = psum_agg.tile([HID_DIM, G], f32, tag="ps")
                        n_mm = int(t_arr[g].sum())
                        if 'aggmm' in skip:
                            nc.tensor.matmul(
                                out=ps[:], lhsT=mw[:, 0, :],
                                rhs=St_dummy[:] if St_dummy is not None
                                else R_f[:].bitcast(f32r),
                                start=True, stop=True)
                        mm_i = 0
                        for k in range(N_CHUNK):
                            ck = wc_tilebase[(wi, k)] - wave_t0
                            jbase = int(t_arr[wg[0]:g, k].sum())
                            for j in range(int(t_arr[g, k])):
                                wt = ck + jbase + j
                                ft = wave_t0 + wt if k == 0 else (
                                    wc_tilebase[(wi, k)] + jbase + j)
                                if 'sbuild' in skip:
                                    St = St_dummy
                                else:
                                    St = spool.tile([P, G], f32r, tag="St")
                                    nc.vector.tensor_scalar(
                                        out=St[:], in0=R_f[:],
                                        scalar1=doff_t[:, ft:ft + 1],
                                        scalar2=None,
                                        op0=mybir.AluOpType.is_equal)
                                if 'aggmm' not in skip:
                                    nc.tensor.matmul(
                                        out=ps[:], lhsT=mw[:, wt, :], rhs=St[:],
                                        start=(mm_i == 0),
                                        stop=(mm_i == n_mm - 1))
                                mm_i += 1
                        # ---- epilogue for group g ----
                        c0 = (g - wg[0]) * G
                        c1 = c0 + G
                        e1 = sbuf.tile([HID_DIM, G], f32, tag="e1")
                        nc.vector.tensor_tensor(out=e1[:], in0=ps[:],
                                                in1=zsw[:, c0:c1],
                                                op=mybir.AluOpType.add)
                        if final:
                            e2 = sbuf.tile([HID_DIM, G], f32r, tag="e2")
                            nc.vector.tensor_tensor(out=e2[:], in0=e1[:],
                                                    in1=dvw[:, c0:c1],
                                                    op=mybir.AluOpType.mult)
                            po = psum_mm2.tile([8, G], f32, tag="po")
                            nc.tensor.matmul(out=po[:], lhsT=W3_t[:],
                                             rhs=e2[:], start=True, stop=True)
                            nc.vector.tensor_scalar(
                                out=zcw[:, c0:c1], in0=po[:],
                                scalar1=b3_t[:, :1],
                                scalar2=None, op0=mybir.AluOpType.add)
                        else:
                            e2 = sbuf.tile([HID_DIM, G], f32, tag="e2")
                            nc.vector.tensor_tensor(out=e2[:], in0=e1[:],
                                                    in1=dvw[:, c0:c1],
                                                    op=mybir.AluOpType.mult)
                            hT = sbuf.tile([HID_DIM, G], f32r, tag="hT")
                            nc.vector.tensor_scalar(
                                out=hT[:], in0=e2[:], scalar1=bias_t[:, :1],
                                scalar2=0.0, op0=mybir.AluOpType.add,
                                op1=mybir.AluOpType.max)
                            if W_next is not None:
                                po = psum_mm2.tile([HID_DIM, G], f32, tag="po")
                                nc.tensor.matmul(out=po[:], lhsT=W_next[:],
                                                 rhs=hT[:], start=True,
                                                 stop=True)
                                nc.vector.tensor_tensor(
                                    out=zcw[:, c0:c1], in0=po[:],
                                    in1=dvw[:, c0:c1],
                                    op=mybir.AluOpType.mult)
                            else:
                                nc.vector.tensor_tensor(
                                    out=zcw[:, c0:c1], in0=hT[:],
                                    in1=dvw[:, c0:c1],
                                    op=mybir.AluOpType.mult)
                    if final:
                        store_wave_fm(zcw, wg, 8, out_d, None)
                    else:
                        store_wave_fm(zcw, wg, HID_DIM, zsh[layer],
                                      zT_out_d)
                        if wi + 1 == len(WAVES) or WAVES[wi + 1][0] != s:
                            sub_allgather(zsh[layer], tables[layer], s)

            agg_layer(1, tables[0], zTd[0], zTd[1], W2_t, b1_t)
            agg_layer(2, tables[1], zTd[1], zTd[0], None, b2_t)
            agg_layer(3, tables[2], zTd[0], None, None, b3_t, final=True)
        stack.close()

    nc.finalize()
    return nc


def _make_in_maps(S, x, W1, b1, W2, b2, W3, b3):
    dinv = S['dinv']
    W3p = np.zeros((HID_DIM, 8), np.float32)
    W3p[:, :OUT_DIM] = W3
    b3p = np.zeros((8, 1), np.float32)
    b3p[:OUT_DIM, 0] = b3
    in_maps = []
    for c in range(N_CORES):
        perm = S['perms'][c]
        xs = np.zeros((NS, IN_DIM), np.float32)
        dv = np.ones(NS, np.float32)
        xs[perm[:NS_RAW]] = x[c * NS_RAW:(c + 1) * NS_RAW]
        dv[perm[:NS_RAW]] = dinv[c * NS_RAW:(c + 1) * NS_RAW]
        in_maps.append({
            "xT": np.ascontiguousarray(xs.T),
            "dinvrep": np.ascontiguousarray(
                np.broadcast_to(dv[None, :], (HID_DIM, NS))),
            "idx": S['idx_arrs'][c],
            "doff": S['doff_arrs'][c],
            "W1": W1, "W2": W2, "W3": W3p,
            "b1": b1.reshape(-1, 1), "b2": b2.reshape(-1, 1), "b3": b3p,
        })
    return in_maps


_LAST = {}


def kernel(x, edge_index, W1, b1, W2, b2, W3, b3):
    x = np.asarray(x, dtype=np.float32)
    W1 = np.asarray(W1, dtype=np.float32)
    W2 = np.asarray(W2, dtype=np.float32)
    W3 = np.asarray(W3, dtype=np.float32)
    b1 = np.asarray(b1, dtype=np.float32)
    b2 = np.asarray(b2, dtype=np.float32)
    b3 = np.asarray(b3, dtype=np.float32)

    S = _host_prep(edge_index)
    nc = _build_program(S)
    in_maps = _make_in_maps(S, x, W1, b1, W2, b2, W3, b3)

    res = run_bass_kernel_spmd(nc, in_maps, core_ids=list(range(N_CORES)))

    _LAST['S'] = S
    _LAST['in_maps'] = in_maps

    out = np.empty((N_NODES, OUT_DIM), np.float32)
    for c in range(N_CORES):
        shard = res.results[c]["out_shard"]       # [NS, 8]
        perm = S['perms'][c]
        out[c * NS_RAW:(c + 1) * NS_RAW] = shard[perm[:NS_RAW], :OUT_DIM]
    return out


def measure_exec_ns(repeats=(1, 5), iters=6, ag_mode='collective', skip=()):
    """Estimate HW exec time by building R-times-repeated variants of the
    full pipeline and differencing pipelined wall-clock."""
    import time
    import jax
    from jax.sharding import Mesh, PartitionSpec, NamedSharding
    from jax.experimental.shard_map import shard_map
    from concourse import bass2jax
    from concourse.bass2jax import _bass_exec_p, install_neuronx_cc_hook

    S, in_maps = _LAST['S'], _LAST['in_maps']
    install_neuronx_cc_hook()
    per_call = {}
    for R in repeats:
        nc = _build_program(S, repeat=R, ag_mode=ag_mode, skip=skip)
        partition_name = (nc.partition_id_tensor.name
                          if nc.partition_id_tensor else None)
        in_names, out_names, out_avals, zero_outs = [], [], [], []
        for alloc in nc.m.functions[0].allocations:
            if not isinstance(alloc, mybir.MemoryLocationSet):
                continue
            name = alloc.memorylocations[0].name
            if alloc.kind == "ExternalInput":
                if name != partition_name:
                    in_names.append(name)
            elif alloc.kind == "ExternalOutput":
                out_names.append(name)
                shape = tuple(alloc.tensor_shape)
                dtype = mybir.dt.np(alloc.dtype)
                out_avals.append(jax.core.ShapedArray(shape, dtype))
                zero_outs.append(np.zeros(shape, dtype))
        all_in = list(in_names) + list(out_names)
        if partition_name:
            all_in.append(partition_name)

        def _body(*args, _nc=nc, _avals=tuple(out_avals), _in=tuple(all_in),
                  _out=tuple(out_names)):
            operands = list(args)
            operands.append(bass2jax.partition_id_tensor())
            return tuple(_bass_exec_p.bind(
                *operands, out_avals=_avals, in_names=_in, out_names=_out,
                lowering_input_output_aliases=(), sim_require_finite=True,
                sim_require_nnan=True, nc=_nc))

        devices = jax.devices()[:N_CORES]
        mesh = Mesh(np.asarray(devices), ("core",))
        nsp = len(in_names) + len(zero_outs)
        sharded = jax.jit(shard_map(
            _body, mesh=mesh, in_specs=(PartitionSpec("core"),) * nsp,
            out_specs=(PartitionSpec("core"),) * len(out_names),
            check_rep=False), keep_unused=True)
        args = [np.concatenate([np.asarray(in_maps[c][n]) for c in
                                range(N_CORES)], axis=0) for n in in_names]
        args += [np.zeros((N_CORES * z.shape[0], *z.shape[1:]), z.dtype)
                 for z in zero_outs]
        sh = NamedSharding(mesh, PartitionSpec("core"))
        args = [jax.device_put(a, sh) for a in args]
        outs = sharded(*args)
        jax.block_until_ready(outs)
        best = None
        for _ in range(iters):
            t0 = time.perf_counter()
            got = [sharded(*args) for _ in range(4)]
            jax.block_until_ready(got)
            dt = (time.perf_counter() - t0) / 4
            best = dt if best is None else min(best, dt)
        per_call[R] = best
    r0, r1 = repeats
    est = (per_call[r1] - per_call[r0]) / (r1 - r0)
    return max(1, int(est * 1e9))



# revision 13
# speedup vs baseline: 4655970.0000x; 1150306.0000x over previous
"""GCN (3-layer, PyG GCNConv semantics) on 8 Trainium2 NeuronCores.

Sharding: nodes are partitioned across the 8 cores by destination id
(graph-parallel). Each core aggregates messages for its own node shard; the
per-layer node features ("tables") are replicated via chunked AllGathers so
every core can gather arbitrary source rows with dma_gather (int16 indices,
so the table is split into 4 row-chunks < 2^15 rows; each chunk is exactly
one sub-AllGather region, letting the collectives overlap the producing
compute).

Math (A_hat = D^-1/2 (A+I) D^-1/2): per layer
    out = dinv * (agg of z) [@ W] + b,  z = dinv * (h @ W)
(W folded before aggregation for layers 1/2, after for layer 3), where
    agg_n = z_n + sum_{e: dst=n} z_src.

Device pipeline per core: L0 computes z1^T per 256-node group
(feature-major); each aggregation layer gathers 2048-row batches from the
table (spread over 4 SWDGE queues), converts them to bf16 on the Activation
engine, builds a selection matrix S[e, j] = (dstoff[e] == j) in bf16 on DVE
(4x mode) and accumulates psum[64, 256] with bf16 matmuls (lhsT = gathered
rows, rhs = S); group epilogues run feature-major, and PE transposes convert
back to node-major shard rows for the next table.

Table storage order within each sub-AllGather region is partition-major
(storage slot = p * nch + chunk_col for node q = chunk_col * 128 + p), so
the node-major stores land as one contiguous run per SBUF partition
(nb*256B descriptors instead of 256B ones).
"""
import sys
sys.path.insert(0, '/opt/trn_rl_repo')

from contextlib import ExitStack

import numpy as np

from concourse import bass, bacc, tile, mybir, library_config
from concourse.bass_utils import run_bass_kernel_spmd
from concourse.masks import make_identity

# ---- problem constants (hardcoded) ----
N_NODES = 100000
IN_DIM, HID_DIM, OUT_DIM = 128, 64, 7
N_CORES = 8
NS_RAW = N_NODES // N_CORES          # 12500 real nodes per core
P = 128
G = 256                              # nodes per aggregation group
NGRP = 49                            # 12544 / 256
NS = NGRP * G                        # 12544 padded shard size
V = NS * N_CORES                     # 100352 table rows
SUB_GRPS = [13, 12, 12, 12]          # groups per sub-AllGather / idx chunk
N_CHUNK = len(SUB_GRPS)
WAVE_SPLITS = {13: [4, 3, 3, 3], 12: [4, 4, 4]}
NI_MAX = 1024                        # dma_gather idxs per instruction cap (ucode limit)
N_QUEUES = 4                         # SWDGE queues for gathers
DMA_SCRATCH = 32768                  # descriptor ring bytes (2048 descs/queue)

f32 = mybir.dt.float32
f32r = mybir.dt.float32r
bf16 = mybir.dt.bfloat16
i16 = mybir.dt.int16

SUB_G0 = np.cumsum([0] + SUB_GRPS)           # group start per sub
SUB_ROWS = [g * G for g in SUB_GRPS]         # shard rows per sub
SUB_R0 = np.cumsum([0] + SUB_ROWS)           # shard row start per sub
SUB_NCH = [r // P for r in SUB_ROWS]         # 128-row chunk cols per sub
CHUNK_ROWS = [r * N_CORES for r in SUB_ROWS]  # table rows per chunk
CHUNK_BASE = np.cumsum([0] + CHUNK_ROWS)
SUB_OF_GROUP = np.concatenate(
    [np.full(n, s, dtype=np.int64) for s, n in enumerate(SUB_GRPS)])

# waves: list of (sub, [groups])
WAVES = []
for s in range(N_CHUNK):
    g0 = SUB_G0[s]
    for w in WAVE_SPLITS[SUB_GRPS[s]]:
        WAVES.append((s, list(range(g0, g0 + w))))
        g0 += w
WGRP_MAX = max(len(wg) for _, wg in WAVES)


def _sigma(loc):
    """Shard-local node index -> storage slot (partition-major within sub)."""
    loc = np.asarray(loc)
    s = SUB_OF_GROUP[loc // G]
    q = loc - SUB_R0[s]
    nch = np.array(SUB_NCH)[s]
    return SUB_R0[s] + (q % P) * nch + q // P


def _host_prep(edge_index):
    """Partition/permute/pad the graph into a static structure shared by all
    cores (cores differ only in input data, not program shape)."""
    src = np.asarray(edge_index[0], dtype=np.int64)
    dst = np.asarray(edge_index[1], dtype=np.int64)
    deg = np.bincount(dst, minlength=N_NODES).astype(np.float64) + 1.0
    dinv = (1.0 / np.sqrt(deg)).astype(np.float32)

    core_of = dst // NS_RAW
    perms = []
    for c in range(N_CORES):
        sel = core_of == c
        dl = (dst[sel] - c * NS_RAW).astype(np.int64)
        cnt = np.bincount(dl, minlength=NS)
        order = np.argsort(-cnt, kind='stable')
        gload = np.zeros(NGRP, dtype=np.int64)
        gfill = np.zeros(NGRP, dtype=np.int64)
        perm = np.empty(NS, dtype=np.int64)
        for node in order:
            cand = np.where(gfill < G)[0]
            gsel = cand[np.argmin(gload[cand])]
            perm[node] = gsel * G + gfill[gsel]
            gfill[gsel] += 1
            gload[gsel] += cnt[node]
        perms.append(perm)

    def trow_chunkidx(nodes):
        """global node id -> (chunk, in-chunk row) [storage order]"""
        c = nodes // NS_RAW
        loc = np.empty(len(nodes), dtype=np.int64)
        for cc in range(N_CORES):
            m = c == cc
            loc[m] = perms[cc][nodes[m] % NS_RAW]
        s = SUB_OF_GROUP[loc // G]
        sig = _sigma(loc) - SUB_R0[s]          # slot within sub
        inrow = c * np.array(SUB_ROWS)[s] + sig
        return s, inrow

    src_ch, src_row = trow_chunkidx(src)

    # per-core (group, chunk) runs
    run_lens = np.zeros((N_CORES, NGRP, N_CHUNK), dtype=np.int64)
    edge_lists = []
    for c in range(N_CORES):
        sel = core_of == c
        rows_c, ch_c = src_row[sel], src_ch[sel]
        d_new = perms[c][(dst[sel] - c * NS_RAW)]
        grp, off = d_new // G, d_new % G
        runs = {}
        for g in range(NGRP):
            gm = grp == g
            for k in range(N_CHUNK):
                m = gm & (ch_c == k)
                runs[(g, k)] = (rows_c[m], off[m])
                run_lens[c, g, k] = m.sum()
        edge_lists.append(runs)

    # static tiles per (group, chunk): max over cores
    t_arr = np.ceil(run_lens.max(axis=0) / P).astype(np.int64)  # [NGRP, N_CHUNK]

    # flat tile order + instruction plan (identical for all cores)
    instr_plan = []   # (wave_idx, chunk, tile_off_in_wavechunk, n_idx, idx_col0)
    tiles_meta = []   # flat: (wave_idx, chunk, group, j)
    wave_tiles = []   # tiles per wave
    idx_cols = 0
    for wi, (s, wg) in enumerate(WAVES):
        wt = 0
        for k in range(N_CHUNK):
            slots = int(t_arr[wg, k].sum()) * P
            done = 0
            while done < slots:
                ni = min(NI_MAX, slots - done)
                instr_plan.append((wi, k, done // P, ni, idx_cols))
                idx_cols += ni // 16
                done += ni
            for g in wg:
                for j in range(int(t_arr[g, k])):
                    tiles_meta.append((wi, k, g, j))
                wt += int(t_arr[g, k])
        wave_tiles.append(wt)
    tt = len(tiles_meta)

    # flat slot base of each (wave, chunk) region
    wc_tilebase = {}
    ti = 0
    for wi, (s, wg) in enumerate(WAVES):
        for k in range(N_CHUNK):
            wc_tilebase[(wi, k)] = ti
            ti += int(t_arr[wg, k].sum())

    idx_arrs, doff_arrs = [], []
    for c in range(N_CORES):
        runs = edge_lists[c]
        flat_idx = np.zeros(tt * P, dtype=np.int16)
        flat_off = np.full(tt * P, -1.0, dtype=np.float32)
        pos = 0
        for (wi, k, g, j) in tiles_meta:
            rows, offs = runs[(g, k)]
            a, b = j * P, min((j + 1) * P, len(rows))
            n = max(0, b - a)
            if n > 0:
                flat_idx[pos:pos + n] = rows[a:b].astype(np.int16)
                flat_off[pos:pos + n] = offs[a:b].astype(np.float32)
            pos += P
        idx_wrapped = np.zeros((P, idx_cols), dtype=np.int16)
        for (wi, k, toff, ni, col0) in instr_plan:
            s0 = (wc_tilebase[(wi, k)] + toff) * P
            blk = flat_idx[s0:s0 + ni].reshape(ni // 16, 16).T
            idx_wrapped[:, col0:col0 + ni // 16] = np.tile(blk, (8, 1))
        idx_arrs.append(idx_wrapped)
        doff_arrs.append(flat_off.reshape(tt, P).T.copy())

    return dict(
        dinv=dinv, perms=perms, t_arr=t_arr, instr_plan=instr_plan,
        tiles_meta=tiles_meta, tt=tt, wave_tiles=wave_tiles,
        wc_tilebase=wc_tilebase, idx_arrs=idx_arrs, doff_arrs=doff_arrs,
        idx_cols=idx_cols,
    )


def _build_program(S, repeat=1, ag_mode='collective', skip=()):
    t_arr, instr_plan, tt, idx_cols = (
        S['t_arr'], S['instr_plan'], S['tt'], S['idx_cols'])
    wave_tiles, wc_tilebase = S['wave_tiles'], S['wc_tilebase']
    mw_tiles_max = max(wave_tiles)

    nc = bacc.Bacc("TRN2", target_bir_lowering=False, debug=False,
                   num_devices=N_CORES,
                   dynamic_dma_scratch_size=DMA_SCRATCH,
                   num_swdge_queues=N_QUEUES)

    xT_d = nc.dram_tensor("xT", [P, NS], f32r, kind="ExternalInput")
    dinvrep_d = nc.dram_tensor("dinvrep", [HID_DIM, NS], f32, kind="ExternalInput")
    idx_d = nc.dram_tensor("idx", [P, idx_cols], i16, kind="ExternalInput")
    doff_d = nc.dram_tensor("doff", [P, tt], f32, kind="ExternalInput")
    W1_d = nc.dram_tensor("W1", [IN_DIM, HID_DIM], f32r, kind="ExternalInput")
    W2_d = nc.dram_tensor("W2", [HID_DIM, HID_DIM], f32r, kind="ExternalInput")
    W3_d = nc.dram_tensor("W3", [HID_DIM, 8], f32r, kind="ExternalInput")
    b1_d = nc.dram_tensor("b1", [HID_DIM, 1], f32, kind="ExternalInput")
    b2_d = nc.dram_tensor("b2", [HID_DIM, 1], f32, kind="ExternalInput")
    b3_d = nc.dram_tensor("b3", [8, 1], f32, kind="ExternalInput")
    out_d = nc.dram_tensor("out_shard", [NS, 8], f32, kind="ExternalOutput")

    nc.gpsimd.load_library(library_config.mlp)

    with tile.TileContext(nc) as tc:
        stack = ExitStack()
        zsh = [tc.tile([NS, HID_DIM], f32r, space="DRAM", name=f"zsh{i}")[0]
               for i in range(3)]
        dramp = stack.enter_context(
            tc.tile_pool(name="dramp", bufs=1, space="DRAM"))
        def alloc_tables(rep):
            return [[dramp.tile([CHUNK_ROWS[k], HID_DIM], f32r,
                                addr_space="Shared",
                                name=f"table{rep}_{i}_{k}",
                                tag=f"table{rep}_{i}_{k}")
                     for k in range(N_CHUNK)] for i in range(3)]
        const = stack.enter_context(tc.tile_pool(name="const", bufs=1))

        R_i = const.tile([P, G], mybir.dt.int32)
        nc.gpsimd.iota(R_i[:], pattern=[[1, G]], base=0, channel_multiplier=0)
        R_f = const.tile([P, G], f32)
        nc.vector.tensor_copy(out=R_f[:], in_=R_i[:])
        R_b = const.tile([P, G], bf16)
        nc.vector.tensor_copy(out=R_b[:], in_=R_i[:])
        ident = const.tile([P, P], f32)
        make_identity(nc, ident[:])
        ident_r = const.tile([P, P], f32r)
        nc.vector.tensor_copy(out=ident_r[:], in_=ident[:])

        St_dummy = None
        if 'sbuild' in skip:
            St_dummy = const.tile([P, G], bf16)
            nc.vector.tensor_copy(out=St_dummy[:], in_=R_f[:])

        W1_t = const.tile([IN_DIM, HID_DIM], f32r)
        nc.sync.dma_start(out=W1_t[:], in_=W1_d[:])
        W2_t = const.tile([HID_DIM, HID_DIM], f32r)
        nc.sync.dma_start(out=W2_t[:], in_=W2_d[:])
        W3_t = const.tile([HID_DIM, 8], f32r)
        nc.sync.dma_start(out=W3_t[:], in_=W3_d[:])
        b1_t = const.tile([HID_DIM, 1], f32)
        nc.sync.dma_start(out=b1_t[:], in_=b1_d[:])
        b2_t = const.tile([HID_DIM, 1], f32)
        nc.sync.dma_start(out=b2_t[:], in_=b2_d[:])
        b3_t = const.tile([8, 1], f32)
        nc.sync.dma_start(out=b3_t[:], in_=b3_d[:])
        idx_t = const.tile([P, idx_cols], i16)
        nc.sync.dma_start(out=idx_t[:], in_=idx_d[:])
        doff_t = const.tile([P, tt], f32)
        nc.sync.dma_start(out=doff_t[:], in_=doff_d[:])

        zTd = [tc.tile([HID_DIM, NS], f32r, space="DRAM", name=f"zTd{i}")[0]
               for i in range(2)]

        sbuf = stack.enter_context(tc.tile_pool(name="sbuf", bufs=3))
        spool = stack.enter_context(tc.tile_pool(name="spool", bufs=6))
        wavep = stack.enter_context(tc.tile_pool(name="wavep", bufs=2))
        znodep = stack.enter_context(tc.tile_pool(name="znodep", bufs=2))
        psum_agg = stack.enter_context(
            tc.tile_pool(name="psum_agg", bufs=3, space="PSUM"))
        psum_mm2 = stack.enter_context(
            tc.tile_pool(name="psum_mm2", bufs=2, space="PSUM"))
        psum_tr = stack.enter_context(
            tc.tile_pool(name="psum_tr", bufs=2, space="PSUM"))

        def load_dvw(wg):
            w0, wn = wg[0] * G, len(wg) * G
            dvw = wavep.tile([HID_DIM, wn], f32, tag="dvw",
                             padded_shape=[HID_DIM, WGRP_MAX * G])
            nc.sync.dma_start(out=dvw[:], in_=dinvrep_d[:, w0:w0 + wn])
            return dvw

        def store_wave_fm(zcw, wi, wg, fdim, node_dram, zT_target):
            """Batch-transpose the feature-major wave tile [fdim, wn] into
            node-major (storage-order) rows of node_dram; also stash
            feature-major into zT_target if given."""
            s = WAVES[wi][0]
            w0, wn = wg[0] * G, len(wg) * G
            r0, rn, nch = SUB_R0[s], SUB_ROWS[s], SUB_NCH[s]
            cc0 = (w0 - r0) // P              # first chunk-col of this wave
            nch_w = wn // P
            sub_nm = node_dram[r0:r0 + rn, :].rearrange(
                "(p c) f -> p c f", p=P)
            for blk0 in range(0, nch_w, 8):
                nb = min(8, nch_w - blk0)
                ptr = psum_tr.tile([P, nb * fdim], f32r, tag="ptr",
                                   padded_shape=[P, 8 * HID_DIM])
                for i in range(nb):
                    nc.tensor.transpose(
                        out=ptr[:, i * fdim:(i + 1) * fdim],
                        in_=zcw[:fdim, (blk0 + i) * P:(blk0 + i + 1) * P],
                        identity=ident_r[:fdim, :fdim])
                zn = znodep.tile([P, nb * fdim], f32r, tag="zn",
                                 padded_shape=[P, 8 * HID_DIM])
                nc.vector.tensor_copy(out=zn[:], in_=ptr[:])
                dst = sub_nm[:, cc0 + blk0: cc0 + blk0 + nb, :]
                src_ap = zn[:] if node_dram is not out_d else zn[:].bitcast(f32)
                nc.scalar.dma_start(
                    out=dst,
                    in_=src_ap.rearrange("p (c f) -> p c f", f=fdim))
            if zT_target is not None:
                nc.scalar.dma_start(out=zT_target[:, w0:w0 + wn], in_=zcw[:])

        def sub_allgather(zsh_t, table_t, s):
            r0, rn = SUB_R0[s], SUB_ROWS[s]
            if ag_mode == 'local':
                nc.scalar.dma_start(out=table_t[s][0:rn, :],
                                    in_=zsh_t[r0:r0 + rn, :])
                return
            nc.gpsimd.collective_compute(
                "AllGather", mybir.AluOpType.bypass,
                replica_groups=[list(range(N_CORES))],
                ins=[zsh_t[r0:r0 + rn, :]],
                outs=[table_t[s][:]])

        gq = [0]  # round-robin gather queue counter (shared across layers)

        for _rep in range(repeat):
            tables = alloc_tables(_rep)
            # ---------- L0: z1 = dinv .* (x @ W1), feature-major ----------
            for wi, (s, wg) in enumerate(WAVES):
                w0, wn = wg[0] * G, len(wg) * G
                xw = wavep.tile([P, wn], f32r, tag="xw",
                                padded_shape=[P, WGRP_MAX * G])
                nc.sync.dma_start(out=xw[:], in_=xT_d[:, w0:w0 + wn])
                dvw = load_dvw(wg)
                zcw = wavep.tile([HID_DIM, wn], f32r, tag="zcw",
                                 padded_shape=[HID_DIM, WGRP_MAX * G])
                for g in wg:
                    c0 = (g - wg[0]) * G
                    ps = psum_agg.tile([HID_DIM, G], f32, tag="ps")
                    nc.tensor.matmul(out=ps[:], lhsT=W1_t[:],
                                     rhs=xw[:, c0:c0 + G],
                                     start=True, stop=True)
                    nc.vector.tensor_tensor(
                        out=zcw[:, c0:c0 + G], in0=ps[:],
                        in1=dvw[:, c0:c0 + G], op=mybir.AluOpType.mult)
                store_wave_fm(zcw, wi, wg, HID_DIM, zsh[0], zTd[0])
                if wi + 1 == len(WAVES) or WAVES[wi + 1][0] != s:
                    sub_allgather(zsh[0], tables[0], s)

            # ---------- aggregation layers ----------
            def agg_layer(layer, table, zT_in, zT_out_d, W_next, bias_t,
                          final=False):
                for wi, (s, wg) in enumerate(WAVES):
                    w0, wn = wg[0] * G, len(wg) * G
                    wtiles = wave_tiles[wi]
                    mw = wavep.tile([P, wtiles, HID_DIM], f32r, tag="mw",
                                    padded_shape=[P, mw_tiles_max, HID_DIM])
                    wave_t0 = wc_tilebase[(wi, 0)]
                    for (wi2, k, toff, ni, col0) in instr_plan:
                        if wi2 != wi or 'gather' in skip:
                            continue
                        ck = wc_tilebase[(wi, k)] - wave_t0
                        nc.gpsimd.dma_gather(
                            out_ap=mw[:, ck + toff: ck + toff + ni // P, :],
                            in_ap=table[k][:],
                            idxs_ap=idx_t[:, col0: col0 + ni // 16],
                            num_idxs=ni, num_idxs_reg=ni, elem_size=HID_DIM,
                            single_packet=True,
                            queue_num=gq[0] % N_QUEUES,
                        )
                        gq[0] += 1
                    if 'gather' in skip:
                        nc.vector.tensor_copy(
                            out=mw[:, 0, :],
                            in_=R_f[:, :HID_DIM].bitcast(f32r))
                    # convert the gathered wave to bf16 on the Act engine
                    mwb = wavep.tile([P, wtiles, HID_DIM], bf16, tag="mwb",
                                     padded_shape=[P, mw_tiles_max, HID_DIM])
                    nc.scalar.copy(out=mwb[:], in_=mw[:])
                    zsw = wavep.tile([HID_DIM, wn], f32r, tag="zsw",
                                     padded_shape=[HID_DIM, WGRP_MAX * G])
                    nc.sync.dma_start(out=zsw[:], in_=zT_in[:, w0:w0 + wn])
                    dvw = load_dvw(wg)
                    if final:
                        zcw = wavep.tile([8, wn], f32r, tag="ocw",
                                         padded_shape=[8, WGRP_MAX * G])
                    else:
                        zcw = wavep.tile([HID_DIM, wn], f32r, tag="zcw",
                                         padded_shape=[HID_DIM, WGRP_MAX * G])
                    for gi, g in enumerate(wg):
                        ps = psum_agg.tile([HID_DIM, G], f32, tag="ps")
                        n_mm = int(t_arr[g].sum())
                        if 'aggmm' in skip:
                            nc.tensor.matmul(
                                out=ps[:], lhsT=mwb[:, 0, :],
                                rhs=St_dummy[:],
                                start=True, stop=True)
                        mm_i = 0
                        for k in range(N_CHUNK):
                            ck = wc_tilebase[(wi, k)] - wave_t0
                            jbase = int(t_arr[wg[0]:g, k].sum())
                            for j in range(int(t_arr[g, k])):
                                wt = ck + jbase + j
                                ft = wave_t0 + wt if k == 0 else (
                                    wc_tilebase[(wi, k)] + jbase + j)
                                if 'sbuild' in skip:
                                    St = St_dummy
                                else:
                                    St = spool.tile([P, G], bf16, tag="St")
                                    nc.vector.tensor_scalar(
                                        out=St[:], in0=R_b[:],
                                        scalar1=doff_t[:, ft:ft + 1],
                                        scalar2=None,
                                        op0=mybir.AluOpType.is_equal)
                                if 'aggmm' not in skip:
                                    nc.tensor.matmul(
                                        out=ps[:], lhsT=mwb[:, wt, :],
                                        rhs=St[:],
                                        start=(mm_i == 0),
                                        stop=(mm_i == n_mm - 1))
                                mm_i += 1
                        # ---- epilogue for group g ----
                        c0 = (g - wg[0]) * G
                        c1 = c0 + G
                        e1 = sbuf.tile([HID_DIM, G], f32, tag="e1")
                        nc.vector.tensor_tensor(out=e1[:], in0=ps[:],
                                                in1=zsw[:, c0:c1],
                                                op=mybir.AluOpType.add)
                        if final:
                            e2 = sbuf.tile([HID_DIM, G], f32r, tag="e2")
                            nc.vector.tensor_tensor(out=e2[:], in0=e1[:],
                                                    in1=dvw[:, c0:c1],
                                                    op=mybir.AluOpType.mult)
                            po = psum_mm2.tile([8, G], f32, tag="po")
                            nc.tensor.matmul(out=po[:], lhsT=W3_t[:],
                                             rhs=e2[:], start=True, stop=True)
                            nc.vector.tensor_scalar(
                                out=zcw[:, c0:c1], in0=po[:],
                                scalar1=b3_t[:, :1],
                                scalar2=None, op0=mybir.AluOpType.add)
                        else:
                            e2 = sbuf.tile([HID_DIM, G], f32, tag="e2")
                            nc.vector.tensor_tensor(out=e2[:], in0=e1[:],
                                                    in1=dvw[:, c0:c1],
                                                    op=mybir.AluOpType.mult)
                            hT = sbuf.tile([HID_DIM, G], f32r, tag="hT")
                            nc.vector.tensor_scalar(
                                out=hT[:], in0=e2[:], scalar1=bias_t[:, :1],
                                scalar2=0.0, op0=mybir.AluOpType.add,
                                op1=mybir.AluOpType.max)
                            if W_next is not None:
                                po = psum_mm2.tile([HID_DIM, G], f32, tag="po")
                                nc.tensor.matmul(out=po[:], lhsT=W_next[:],
                                                 rhs=hT[:], start=True,
                                                 stop=True)
                                nc.vector.tensor_tensor(
                                    out=zcw[:, c0:c1], in0=po[:],
                                    in1=dvw[:, c0:c1],
                                    op=mybir.AluOpType.mult)
                            else:
                                nc.vector.tensor_tensor(
                                    out=zcw[:, c0:c1], in0=hT[:],
                                    in1=dvw[:, c0:c1],
                                    op=mybir.AluOpType.mult)
                    if final:
                        store_wave_fm(zcw, wi, wg, 8, out_d, None)
                    else:
                        store_wave_fm(zcw, wi, wg, HID_DIM, zsh[layer],
                                      zT_out_d)
                        if wi + 1 == len(WAVES) or WAVES[wi + 1][0] != s:
                            sub_allgather(zsh[layer], tables[layer], s)

            agg_layer(1, tables[0], zTd[0], zTd[1], W2_t, b1_t)
            agg_layer(2, tables[1], zTd[1], zTd[0], None, b2_t)
            agg_layer(3, tables[2], zTd[0], None, None, b3_t, final=True)
        stack.close()

    nc.finalize()
    return nc


def _make_in_maps(S, x, W1, b1, W2, b2, W3, b3):
    dinv = S['dinv']
    W3p = np.zeros((HID_DIM, 8), np.float32)
    W3p[:, :OUT_DIM] = W3
    b3p = np.zeros((8, 1), np.float32)
    b3p[:OUT_DIM, 0] = b3
    in_maps = []
    for c in range(N_CORES):
        perm = S['perms'][c]
        xs = np.zeros((NS, IN_DIM), np.float32)
        dv = np.ones(NS, np.float32)
        xs[perm[:NS_RAW]] = x[c * NS_RAW:(c + 1) * NS_RAW]
        dv[perm[:NS_RAW]] = dinv[c * NS_RAW:(c + 1) * NS_RAW]
        in_maps.append({
            "xT": np.ascontiguousarray(xs.T),
            "dinvrep": np.ascontiguousarray(
                np.broadcast_to(dv[None, :], (HID_DIM, NS))),
            "idx": S['idx_arrs'][c],
            "doff": S['doff_arrs'][c],
            "W1": W1, "W2": W2, "W3": W3p,
            "b1": b1.reshape(-1, 1), "b2": b2.reshape(-1, 1), "b3": b3p,
        })
    return in_maps


_LAST = {}


def kernel(x, edge_index, W1, b1, W2, b2, W3, b3):
    x = np.asarray(x, dtype=np.float32)
    W1 = np.asarray(W1, dtype=np.float32)
    W2 = np.asarray(W2, dtype=np.float32)
    W3 = np.asarray(W3, dtype=np.float32)
    b1 = np.asarray(b1, dtype=np.float32)
    b2 = np.asarray(b2, dtype=np.float32)
    b3 = np.asarray(b3, dtype=np.float32)

    S = _host_prep(edge_index)
    nc = _build_program(S)
    in_maps = _make_in_maps(S, x, W1, b1, W2, b2, W3, b3)

    res = run_bass_kernel_spmd(nc, in_maps, core_ids=list(range(N_CORES)))

    _LAST['S'] = S
    _LAST['in_maps'] = in_maps

    sig = _sigma(np.arange(NS))              # loc -> out_shard row
    out = np.empty((N_NODES, OUT_DIM), np.float32)
    for c in range(N_CORES):
        shard = res.results[c]["out_shard"]       # [NS, 8]
        perm = S['perms'][c]
        out[c * NS_RAW:(c + 1) * NS_RAW] = shard[sig[perm[:NS_RAW]], :OUT_DIM]
    return out


def measure_exec_ns(repeats=(1, 5), iters=6, ag_mode='collective', skip=()):
    """Estimate HW exec time by building R-times-repeated variants of the
    full pipeline and differencing pipelined wall-clock."""
    import time
    import jax
    from jax.sharding import Mesh, PartitionSpec, NamedSharding
    from jax.experimental.shard_map import shard_map
    from concourse import bass2jax
    from concourse.bass2jax import _bass_exec_p, install_neuronx_cc_hook

    S, in_maps = _LAST['S'], _LAST['in_maps']
    install_neuronx_cc_hook()
    per_call = {}
    for R in repeats:
        nc = _build_program(S, repeat=R, ag_mode=ag_mode, skip=skip)
        partition_name = (nc.partition_id_tensor.name
                          if nc.partition_id_tensor else None)
        in_names, out_names, out_avals, zero_outs = [], [], [], []
        for alloc in nc.m.functions[0].allocations:
            if not isinstance(alloc, mybir.MemoryLocationSet):
                continue
            name = alloc.memorylocations[0].name
            if alloc.kind == "ExternalInput":
                if name != partition_name:
                    in_names.append(name)
            elif alloc.kind == "ExternalOutput":
                out_names.append(name)
                shape = tuple(alloc.tensor_shape)
                dtype = mybir.dt.np(alloc.dtype)
                out_avals.append(jax.core.ShapedArray(shape, dtype))
                zero_outs.append(np.zeros(shape, dtype))
        all_in = list(in_names) + list(out_names)
        if partition_name:
            all_in.append(partition_name)

        def _body(*args, _nc=nc, _avals=tuple(out_avals), _in=tuple(all_in),
                  _out=tuple(out_names)):
            operands = list(args)
            operands.append(bass2jax.partition_id_tensor())
            return tuple(_bass_exec_p.bind(
                *operands, out_avals=_avals, in_names=_in, out_names=_out,
                lowering_input_output_aliases=(), sim_require_finite=True,
                sim_require_nnan=True, nc=_nc))

        devices = jax.devices()[:N_CORES]
        mesh = Mesh(np.asarray(devices), ("core",))
        nsp = len(in_names) + len(zero_outs)
        sharded = jax.jit(shard_map(
            _body, mesh=mesh, in_specs=(PartitionSpec("core"),) * nsp,
            out_specs=(PartitionSpec("core"),) * len(out_names),
            check_rep=False), keep_unused=True)
        args = [np.concatenate([np.asarray(in_maps[c][n]) for c in
                                range(N_CORES)], axis=0) for n in in_names]
        args += [np.zeros((N_CORES * z.shape[0], *z.shape[1:]), z.dtype)
                 for z in zero_outs]
        sh = NamedSharding(mesh, PartitionSpec("core"))
        args = [jax.device_put(a, sh) for a in args]
        outs = sharded(*args)
        jax.block_until_ready(outs)
        best = None
        for _ in range(iters):
            t0 = time.perf_counter()
            got = [sharded(*args) for _ in range(4)]
            jax.block_until_ready(got)
            dt = (time.perf_counter() - t0) / 4
            best = dt if best is None else min(best, dt)
        per_call[R] = best
    r0, r1 = repeats
    est = (per_call[r1] - per_call[r0]) / (r1 - r0)
    return max(1, int(est * 1e9))
